# revision 1
# baseline (speedup 1.0000x reference)
"""KNN attention kernel for 8 Trainium2 NeuronCores.

Sharding: (batch, head-group) data parallel. Core c handles batch c//2 and
heads (c%2)*8 .. (c%2)*8+8.  Each core computes a partial final projection
(its 512 attention-output channels x Wc^T slice); the host sums the two
partials per batch.  All activations are fed pre-transposed ([d, l]) so every
contraction has its K dim on partitions without on-device transposes.
"""

import sys

sys.path.insert(0, "/opt/trn_rl_repo")

import numpy as np

B, L, D, DH, H = 4, 1024, 1024, 64, 16
HPG = 8          # heads per core
CPG = HPG * DH   # channels per core (512)

_CACHE = {}


def _split_sync_waits(nc, mybir, max_waits=1):
    """This container's walrus rejects >1 sync wait per instruction; spill
    extras onto same-engine NOPs placed immediately before."""
    for fn in nc.m.functions:
        for bb in fn.blocks:
            old = list(bb.instructions)
            new_insts = []
            changed = False
            for inst in old:
                si = inst.sync_info
                if si is not None and len(si.on_wait) > max_waits:
                    waits = list(si.on_wait)
                    extra, keep = waits[:-max_waits], waits[-max_waits:]
                    k = 0
                    while extra:
                        chunk, extra = extra[:max_waits], extra[max_waits:]
                        nop = mybir.InstNoOp(
                            name=f"{inst.name}_ws{k}", ins=[], outs=[])
                        nop.engine = inst.engine
                        nop.sync_info = mybir.SyncInfo(
                            on_wait=chunk, on_update=[])
                        nc.register_instruction(nop)
                        new_insts.append(nop)
                        k += 1
                    inst.sync_info = mybir.SyncInfo(
                        on_wait=keep, on_update=list(si.on_update))
                    changed = True
                new_insts.append(inst)
            if changed:
                bb.instructions = new_insts


def _build_nc():
    import concourse.bass as bass
    import concourse.mybir as mybir
    import concourse.tile as tile
    from concourse.masks import make_identity

    f32 = mybir.dt.float32
    bf16 = mybir.dt.bfloat16
    u32 = mybir.dt.uint32
    Exp = mybir.ActivationFunctionType.Exp
    Square = mybir.ActivationFunctionType.Square
    mul_op = mybir.AluOpType.mult

    nc = bass.Bass("TRN2", target_bir_lowering=False, debug=False)

    qT = nc.dram_tensor("qT", [D, L], f32, kind="ExternalInput")
    kvT = nc.dram_tensor("kvT", [D, L], f32, kind="ExternalInput")
    wqT = nc.dram_tensor("wqT", [D, CPG], f32, kind="ExternalInput")
    wkvT = nc.dram_tensor("wkvT", [D, 2 * DH], f32, kind="ExternalInput")
    wcT = nc.dram_tensor("wcT", [CPG, D], bf16, kind="ExternalInput")
    gates = nc.dram_tensor("gates", [128, 2], f32, kind="ExternalInput")
    y = nc.dram_tensor("y", [L, D], f32, kind="ExternalOutput")

    k_nat_d = nc.dram_tensor("k_nat_d", [L, DH], bf16)
    v_ret_d = nc.dram_tensor("v_ret_d", [L, DH], bf16)

    with tile.TileContext(nc) as tc:
        with (
            tc.tile_pool(name="persist", bufs=1) as pw,
            tc.tile_pool(name="psbig", bufs=2, space="PSUM") as ps_big,
            tc.tile_pool(name="psav", bufs=2, space="PSUM") as ps_av,
            tc.tile_pool(name="pssm", bufs=2, space="PSUM") as ps_sm,
        ):
            ident_bf = pw.tile([128, 128], bf16)
            make_identity(nc, ident_bf[:])
            gates_sb = pw.tile([128, 2], f32)
            nc.sync.dma_start(out=gates_sb[:], in_=gates[:])
            ones_sb = pw.tile([128, 64], f32)
            nc.vector.memset(ones_sb[:], 1.0)

            wc_sb = pw.tile([128, 4, D], bf16)
            for cc in range(4):
                nc.sync.dma_start(
                    out=wc_sb[:, cc, :], in_=wcT[cc * 128:(cc + 1) * 128, :])

            qpT_f = pw.tile([128, 4, L], f32)     # [c=512, i] c=cc*128+p
            qpT_b = pw.tile([128, 4, L], bf16)
            kT2_f = pw.tile([128, L], f32)        # rows 0:64 kT, 64:128 dup
            kT2_b = pw.tile([128, L], bf16)
            vloc_T = pw.tile([128, L], bf16)      # rows 64:128 used
            vret_T = pw.tile([128, L], bf16)      # rows 64:128 used
            vloc_nat = pw.tile([128, 8, DH + 1], bf16)
            attnT = pw.tile([128, 4, L], bf16)    # [c=512, i]

            # ---------------- phase A: projections ----------------
            with tc.tile_pool(name="load", bufs=1) as pl, \
                 tc.tile_pool(name="worka", bufs=2) as wa:
                qT_sb = pl.tile([128, 8, L], f32)
                kvT_sb = pl.tile([128, 8, L], f32)
                wq_sb = pl.tile([128, 8, CPG], f32)
                wkv_sb = pl.tile([128, 8, 2 * DH], f32)
                for kc in range(8):
                    nc.sync.dma_start(
                        out=qT_sb[:, kc, :], in_=qT[kc * 128:(kc + 1) * 128, :])
                    nc.sync.dma_start(
                        out=kvT_sb[:, kc, :], in_=kvT[kc * 128:(kc + 1) * 128, :])
                    nc.sync.dma_start(
                        out=wq_sb[:, kc, :], in_=wqT[kc * 128:(kc + 1) * 128, :])
                    nc.sync.dma_start(
                        out=wkv_sb[:, kc, :], in_=wkvT[kc * 128:(kc + 1) * 128, :])

                # kv projection: kvpT[cc, i] (cc = 0..128 = 2*DH)
                kvp_sb = wa.tile([128, L], f32, tag="kvp")
                for ic in range(2):
                    ps = ps_av.tile([128, 512], f32, tag="av")
                    for kc in range(8):
                        nc.tensor.matmul(
                            ps[:],
                            lhsT=wkv_sb[:, kc, :],
                            rhs=kvT_sb[:, kc, ic * 512:(ic + 1) * 512],
                            start=(kc == 0), stop=(kc == 7))
                    nc.vector.tensor_copy(
                        out=kvp_sb[:, ic * 512:(ic + 1) * 512], in_=ps[:])

                # l2 norm over seq dim (free) + 1/sqrt(dh) fold into k rows
                sqd = wa.tile([128, L], f32, tag="sqd")
                ssum = wa.tile([128, 1], f32, tag="ss")
                nc.scalar.activation(
                    out=sqd[:], in_=kvp_sb[:], func=Square, accum_out=ssum[:])
                snorm = wa.tile([128, 1], f32, tag="sn")
                nc.scalar.sqrt(out=snorm[:], in_=ssum[:])
                rn = wa.tile([128, 1], f32, tag="rn")
                nc.vector.reciprocal(out=rn[:], in_=snorm[:])
                nc.scalar.mul(out=rn[0:64, :], in_=rn[0:64, :], mul=0.125)

                kvn = wa.tile([128, L], f32, tag="kvn")
                nc.vector.tensor_scalar_mul(kvn[:], kvp_sb[:], rn[:, 0:1])

                nc.vector.tensor_copy(out=kT2_f[0:64, :], in_=kvn[0:64, :])
                nc.vector.tensor_copy(out=kT2_b[0:64, :], in_=kvn[0:64, :])
                nc.sync.dma_start(out=kT2_f[64:128, :], in_=kT2_f[0:64, :])
                nc.sync.dma_start(out=kT2_b[64:128, :], in_=kT2_b[0:64, :])

                # gate-folded value copies (rows 64:128)
                nc.vector.tensor_scalar_mul(
                    vloc_T[64:128, :], kvn[64:128, :], gates_sb[64:128, 1:2])
                nc.vector.tensor_scalar_mul(
                    vret_T[64:128, :], kvn[64:128, :], gates_sb[64:128, 0:1])

                # natural-layout copies: vloc (sbuf, +ones col), vret/k (dram)
                nc.vector.memset(vloc_nat[:, :, DH:DH + 1], 1.0)
                for jc in range(8):
                    tp = ps_sm.tile([128, 128], bf16, tag="sm")
                    nc.tensor.transpose(
                        out=tp[:, 0:64],
                        in_=vloc_T[64:128, jc * 128:(jc + 1) * 128],
                        identity=ident_bf[64:128, 64:128])
                    nc.vector.tensor_copy(
                        out=vloc_nat[:, jc, 0:DH], in_=tp[:, 0:64])

                    tp2 = ps_sm.tile([128, 128], bf16, tag="sm")
                    nc.tensor.transpose(
                        out=tp2[:, 0:64],
                        in_=vret_T[64:128, jc * 128:(jc + 1) * 128],
                        identity=ident_bf[64:128, 64:128])
                    vr = wa.tile([128, DH], bf16, tag="vr")
                    nc.vector.tensor_copy(out=vr[:], in_=tp2[:, 0:64])
                    nc.sync.dma_start(
                        out=v_ret_d[jc * 128:(jc + 1) * 128, :], in_=vr[:])

                    tp3 = ps_sm.tile([128, 128], bf16, tag="sm")
                    nc.tensor.transpose(
                        out=tp3[:, 0:64],
                        in_=kT2_b[0:64, jc * 128:(jc + 1) * 128],
                        identity=ident_bf[0:64, 0:64])
                    kn = wa.tile([128, DH], bf16, tag="kn")
                    nc.vector.tensor_copy(out=kn[:], in_=tp3[:, 0:64])
                    nc.sync.dma_start(
                        out=k_nat_d[jc * 128:(jc + 1) * 128, :], in_=kn[:])

                # q projection qpT[c, i]
                for cc in range(4):
                    for ic in range(2):
                        ps = ps_av.tile([128, 512], f32, tag="av")
                        for kc in range(8):
                            nc.tensor.matmul(
                                ps[:],
                                lhsT=wq_sb[:, kc, cc * 128:(cc + 1) * 128],
                                rhs=qT_sb[:, kc, ic * 512:(ic + 1) * 512],
                                start=(kc == 0), stop=(kc == 7))
                        sl = slice(ic * 512, (ic + 1) * 512)
                        nc.vector.tensor_copy(out=qpT_f[:, cc, sl], in_=ps[:])
                        nc.scalar.copy(out=qpT_b[:, cc, sl], in_=ps[:])

            # ---------------- phase B: per-head attention ----------------
            with tc.tile_pool(name="head", bufs=2) as ph:
                for h in range(HPG):
                    pb = (h % 2) * 64
                    cc = h // 2
                    qh_f = qpT_f[pb:pb + 64, cc, :]     # [64, L] f32 view
                    qh_b = qpT_b[pb:pb + 64, cc, :]     # [64, L] bf16 view

                    # --- scores S[i, j] (fp32) + argmax ---
                    idx8 = ph.tile([128, 8, 8], u32, tag="idx")
                    for qi in range(8):
                        s_ps = ps_big.tile([128, 1024], f32, tag="sbig")
                        for jh in range(2):
                            nc.tensor.matmul(
                                s_ps[:, jh * 512:(jh + 1) * 512],
                                lhsT=qh_f[:, qi * 128:(qi + 1) * 128],
                                rhs=kT2_f[pb:pb + 64, jh * 512:(jh + 1) * 512],
                                start=True, stop=True)
                        ssb = ph.tile([128, 1024], f32, tag="ssb")
                        nc.vector.tensor_copy(out=ssb[:], in_=s_ps[:])
                        m8 = ph.tile([128, 8], f32, tag="m8")
                        nc.vector.max(out=m8[:], in_=ssb[:])
                        nc.vector.max_index(
                            out=idx8[:, qi, :], in_max=m8[:], in_values=ssb[:])

                    # --- local: E = exp(S^T) ---
                    E1 = ph.tile([128, 8, 1024], bf16, tag="E1")
                    for jc in range(8):
                        st_ps = ps_big.tile([128, 1024], f32, tag="sbig")
                        for ih in range(2):
                            nc.tensor.matmul(
                                st_ps[:, ih * 512:(ih + 1) * 512],
                                lhsT=kT2_b[pb:pb + 64, jc * 128:(jc + 1) * 128],
                                rhs=qh_b[:, ih * 512:(ih + 1) * 512],
                                start=True, stop=True)
                        nc.scalar.activation(
                            out=E1[:, jc, :], in_=st_ps[:], func=Exp)

                    # --- gather retrieved k/v rows; build rkT (dup halves) ---
                    rkT = ph.tile([128, 1024], bf16, tag="rkT")
                    rv_nat = ph.tile([128, 8, DH + 1], bf16, tag="rvn")
                    nc.vector.memset(rv_nat[:, :, DH:DH + 1], 1.0)
                    for qi in range(8):
                        rk = ph.tile([128, DH], bf16, tag="rk")
                        nc.gpsimd.indirect_dma_start(
                            out=rk[:], out_offset=None,
                            in_=k_nat_d[:],
                            in_offset=bass.IndirectOffsetOnAxis(
                                ap=idx8[:, qi, 0:1], axis=0))
                        nc.gpsimd.indirect_dma_start(
                            out=rv_nat[:, qi, 0:DH], out_offset=None,
                            in_=v_ret_d[:],
                            in_offset=bass.IndirectOffsetOnAxis(
                                ap=idx8[:, qi, 0:1], axis=0))
                        tp = ps_sm.tile([128, 128], bf16, tag="sm")
                        nc.tensor.transpose(
                            out=tp[0:64, :], in_=rk[:],
                            identity=ident_bf[:, :])
                        nc.vector.tensor_copy(
                            out=rkT[0:64, qi * 128:(qi + 1) * 128],
                            in_=tp[0:64, :])
                    nc.sync.dma_start(
                        out=rkT[64:128, :], in_=rkT[0:64, :])

                    # --- retrieval: E2 = exp(S2^T) ---
                    E2 = ph.tile([128, 8, 1024], bf16, tag="E2")
                    for jc in range(8):
                        st_ps = ps_big.tile([128, 1024], f32, tag="sbig")
                        for ih in range(2):
                            nc.tensor.matmul(
                                st_ps[:, ih * 512:(ih + 1) * 512],
                                lhsT=rkT[pb:pb + 64, jc * 128:(jc + 1) * 128],
                                rhs=qh_b[:, ih * 512:(ih + 1) * 512],
                                start=True, stop=True)
                        nc.scalar.activation(
                            out=E2[:, jc, :], in_=st_ps[:], func=Exp)

                    # --- weighted sums + normalize + combine ---
                    attn_h = ph.tile([64, 1024], bf16, tag="ath")
                    for ic in range(2):
                        isl = slice(ic * 512, (ic + 1) * 512)
                        avL = ps_av.tile([65, 512], f32, tag="av")
                        avR = ps_av.tile([65, 512], f32, tag="av")
                        for jc in range(8):
                            nc.tensor.matmul(
                                avL[:], lhsT=vloc_nat[:, jc, :],
                                rhs=E1[:, jc, isl],
                                start=(jc == 0), stop=(jc == 7))
                        for jc in range(8):
                            nc.tensor.matmul(
                                avR[:], lhsT=rv_nat[:, jc, :],
                                rhs=E2[:, jc, isl],
                                start=(jc == 0), stop=(jc == 7))
                        rL = ph.tile([65, 512], f32, tag="rL")
                        rR = ph.tile([65, 512], f32, tag="rR")
                        nc.vector.reciprocal(out=rL[64:65, :], in_=avL[64:65, :])
                        nc.vector.reciprocal(out=rR[64:65, :], in_=avR[64:65, :])
                        bcL = ps_sm.tile([64, 512], f32, tag="sm")
                        bcR = ps_sm.tile([64, 512], f32, tag="sm")
                        nc.tensor.matmul(
                            bcL[:], lhsT=ones_sb[64:65, :], rhs=rL[64:65, :],
                            start=True, stop=True)
                        nc.tensor.matmul(
                            bcR[:], lhsT=ones_sb[64:65, :], rhs=rR[64:65, :],
                            start=True, stop=True)
                        bcLs = ph.tile([64, 512], f32, tag="bcLs")
                        bcRs = ph.tile([64, 512], f32, tag="bcRs")
                        nc.vector.tensor_copy(out=bcLs[:], in_=bcL[:])
                        nc.vector.tensor_copy(out=bcRs[:], in_=bcR[:])
                        bLs = ph.tile([64, 512], f32, tag="bLs")
                        bRs = ph.tile([64, 512], f32, tag="bRs")
                        nc.vector.tensor_tensor(
                            out=bLs[:], in0=avL[0:64, :], in1=bcLs[:], op=mul_op)
                        nc.vector.tensor_tensor(
                            out=bRs[:], in0=avR[0:64, :], in1=bcRs[:], op=mul_op)
                        nc.vector.tensor_add(
                            out=attn_h[:, isl], in0=bLs[:], in1=bRs[:])
                    nc.sync.dma_start(
                        out=attnT[pb:pb + 64, cc, :], in_=attn_h[:])

                # ---------------- phase C: output projection ----------------
                for mi in range(8):
                    for nh in range(2):
                        y_ps = ps_av.tile([128, 512], f32, tag="av")
                        for cc2 in range(4):
                            nc.tensor.matmul(
                                y_ps[:],
                                lhsT=attnT[:, cc2, mi * 128:(mi + 1) * 128],
                                rhs=wc_sb[:, cc2, nh * 512:(nh + 1) * 512],
                                start=(cc2 == 0), stop=(cc2 == 3))
                        y_sb = ph.tile([128, 512], f32, tag="ysb")
                        nc.vector.tensor_copy(out=y_sb[:], in_=y_ps[:])
                        nc.sync.dma_start(
                            out=y[mi * 128:(mi + 1) * 128,
                                  nh * 512:(nh + 1) * 512],
                            in_=y_sb[:])

    _split_sync_waits(nc, mybir, max_waits=1)
    return nc


def kernel(q, kv, Wq, Wkv, Wc, bias):
    import ml_dtypes
    from concourse.bass_utils import run_bass_kernel_spmd

    if "nc" not in _CACHE:
        _CACHE["nc"] = _build_nc()
    nc = _CACHE["nc"]

    g = 1.0 / (1.0 + np.exp(-bias.astype(np.float64)))
    gates = np.stack([g, 1.0 - g], axis=1).astype(np.float32)   # [64, 2]
    gates = np.tile(gates, (2, 1))                               # [128, 2]

    wkvT = np.ascontiguousarray(Wkv.T)                           # [D, 128]
    in_maps = []
    for c in range(8):
        bi, hg = c // 2, c % 2
        in_maps.append({
            "qT": np.ascontiguousarray(q[bi].T),
            "kvT": np.ascontiguousarray(kv[bi].T),
            "wqT": np.ascontiguousarray(Wq[hg * CPG:(hg + 1) * CPG, :].T),
            "wkvT": wkvT,
            "wcT": np.ascontiguousarray(
                Wc[:, hg * CPG:(hg + 1) * CPG].T).astype(ml_dtypes.bfloat16),
            "gates": gates,
        })
    res = run_bass_kernel_spmd(nc, in_maps, list(range(8)))
    out = np.empty((B, L, D), np.float32)
    for bi in range(B):
        out[bi] = res.results[2 * bi]["y"] + res.results[2 * bi + 1]["y"]
    return out



# revision 4
# speedup vs baseline: 2.6798x; 2.6798x over previous
"""KNN attention kernel for 8 Trainium2 NeuronCores.

Sharding: (batch, head-group) data parallel — core c handles batch c//2 and
heads (c%2)*8..(c%2)*8+8.  To minimize axon-tunnel traffic (the wall-clock
bottleneck), each core uploads a unique 1/8 shard of the inputs; on-device
AllGathers redistribute: pair gathers [[0,1],[2,3],..] for per-batch data and
even/odd gathers [[0,2,4,6],[1,3,5,7]] for per-head-group weight halves.

Precision split: the retrieval argmax is exquisitely sensitive to score
noise (bf16 inputs cause ~350 index flips → 4e-2 rel err), so the score
path ships in f32: q, Wq, and a host-computed kT2 = 0.125*l2norm(k).T
(0.5 GFLOP on host).  The value path (kv, Wkv, Wc) and the output y ship
in bf16 (~5e-3 rel err total).  Partial final projections are
pair-ReduceScattered on device so each core outputs a disjoint bf16
[512, 1024] slice of y.
"""

import sys

sys.path.insert(0, "/opt/trn_rl_repo")

import numpy as np

B, L, D, DH, H = 4, 1024, 1024, 64, 16
HPG = 8          # heads per core
CPG = HPG * DH   # channels per core (512)

# f32 blob: pair content rows [q 0:1024 | kT2 1024:1088 | gates 1088:1089 |
# pad to 1154], eo content rows [wqT half flat 0:512]
P32_Q, P32_KT, P32_G = 0, 1024, 1088
P32_R, P32_SH = 1154, 577
E32_R, E32_SH = 512, 128
B32_R = P32_SH + E32_SH  # 705

# bf16 blob: pair content rows [kv 0:1024 | Wkv 1024:1152], eo content
# rows [wcT half 0:512]
P16_KV, P16_WKV = 0, 1024
P16_R, P16_SH = 1152, 576
E16_R, E16_SH = 512, 128
B16_R = P16_SH + E16_SH  # 704

_CACHE = {}


def _split_sync_waits(nc, mybir, max_waits=1):
    """This container's walrus rejects >1 sync wait per instruction; spill
    extras onto same-engine NOPs placed immediately before."""
    for fn in nc.m.functions:
        for bb in fn.blocks:
            old = list(bb.instructions)
            new_insts = []
            changed = False
            for inst in old:
                si = inst.sync_info
                if si is not None and len(si.on_wait) > max_waits:
                    waits = list(si.on_wait)
                    extra, keep = waits[:-max_waits], waits[-max_waits:]
                    k = 0
                    while extra:
                        chunk, extra = extra[:max_waits], extra[max_waits:]
                        nop = mybir.InstNoOp(
                            name=f"{inst.name}_ws{k}", ins=[], outs=[])
                        nop.engine = inst.engine
                        nop.sync_info = mybir.SyncInfo(
                            on_wait=chunk, on_update=[])
                        nc.register_instruction(nop)
                        new_insts.append(nop)
                        k += 1
                    inst.sync_info = mybir.SyncInfo(
                        on_wait=keep, on_update=list(si.on_update))
                    changed = True
                new_insts.append(inst)
            if changed:
                bb.instructions = new_insts


def _build_nc():
    import concourse.bass as bass
    import concourse.mybir as mybir
    import concourse.tile as tile
    from concourse.masks import make_identity

    f32 = mybir.dt.float32
    bf16 = mybir.dt.bfloat16
    u32 = mybir.dt.uint32
    Exp = mybir.ActivationFunctionType.Exp
    Square = mybir.ActivationFunctionType.Square
    mul_op = mybir.AluOpType.mult

    nc = bass.Bass("TRN2", target_bir_lowering=False, debug=False,
                   num_devices=8)

    blob32 = nc.dram_tensor("blob32", [B32_R, 1024], f32,
                            kind="ExternalInput")
    blob16 = nc.dram_tensor("blob16", [B16_R, 1024], bf16,
                            kind="ExternalInput")
    y = nc.dram_tensor("y", [512, D], bf16, kind="ExternalOutput")

    k_nat_d = nc.dram_tensor("k_nat_d", [L, DH], bf16)
    v_ret_d = nc.dram_tensor("v_ret_d", [L, DH], bf16)

    PAIRS = [[0, 1], [2, 3], [4, 5], [6, 7]]
    EODD = [[0, 2, 4, 6], [1, 3, 5, 7]]

    with tile.TileContext(nc) as tc:
        with (
            tc.tile_pool(name="dram", bufs=1, space="DRAM") as dp,
            tc.tile_pool(name="persist", bufs=1) as pw,
            tc.tile_pool(name="psbig", bufs=2, space="PSUM") as ps_big,
            tc.tile_pool(name="psav", bufs=2, space="PSUM") as ps_av,
            tc.tile_pool(name="pssm", bufs=2, space="PSUM") as ps_sm,
        ):
            # ---------------- phase 0: gather shards ----------------
            p32_in = dp.tile([P32_SH, 1024], f32)
            p32_full = dp.tile([P32_R, 1024], f32)
            e32_in = dp.tile([E32_SH, 1024], f32)
            e32_full = dp.tile([E32_R, 1024], f32)
            p16_in = dp.tile([P16_SH, 1024], bf16)
            p16_full = dp.tile([P16_R, 1024], bf16)
            e16_in = dp.tile([E16_SH, 1024], bf16)
            e16_full = dp.tile([E16_R, 1024], bf16)
            ypart = dp.tile([L, D], bf16)
            yhalf = dp.tile([512, D], bf16)

            nc.sync.dma_start(out=p32_in[:], in_=blob32[0:P32_SH, :])
            nc.sync.dma_start(out=e32_in[:], in_=blob32[P32_SH:B32_R, :])
            nc.sync.dma_start(out=p16_in[:], in_=blob16[0:P16_SH, :])
            nc.sync.dma_start(out=e16_in[:], in_=blob16[P16_SH:B16_R, :])
            nc.gpsimd.collective_compute(
                "AllGather", mybir.AluOpType.bypass, replica_groups=PAIRS,
                ins=[p32_in[:].opt()], outs=[p32_full[:].opt()])
            nc.gpsimd.collective_compute(
                "AllGather", mybir.AluOpType.bypass, replica_groups=EODD,
                ins=[e32_in[:].opt()], outs=[e32_full[:].opt()])
            nc.gpsimd.collective_compute(
                "AllGather", mybir.AluOpType.bypass, replica_groups=PAIRS,
                ins=[p16_in[:].opt()], outs=[p16_full[:].opt()])
            nc.gpsimd.collective_compute(
                "AllGather", mybir.AluOpType.bypass, replica_groups=EODD,
                ins=[e16_in[:].opt()], outs=[e16_full[:].opt()])

            ident_bf = pw.tile([128, 128], bf16)
            make_identity(nc, ident_bf[:])
            ident_f = pw.tile([128, 128], f32)
            make_identity(nc, ident_f[:])
            ones_sb = pw.tile([128, 64], f32)
            nc.vector.memset(ones_sb[:], 1.0)

            # gates [128, 2] f32 <- one interleaved f32 row
            gates_sb = pw.tile([128, 2], f32)
            nc.sync.dma_start(
                out=gates_sb[:], in_=p32_full[P32_G:P32_G + 1, 0:256])

            # wc_sb[:, cc, :] = wcT rows (hg half, device-invariant layout)
            wc_sb = pw.tile([128, 4, D], bf16)
            for cc in range(4):
                nc.sync.dma_start(
                    out=wc_sb[:, cc, :],
                    in_=e16_full[cc * 128:(cc + 1) * 128, :])

            qpT_f = pw.tile([128, 4, L], f32)     # [c=512, i] c=cc*128+p
            qpT_b = pw.tile([128, 4, L], bf16)
            kT2_f = pw.tile([128, L], f32)        # rows 0:64 kT, 64:128 dup
            kT2_b = pw.tile([128, L], bf16)
            vloc_T = pw.tile([128, L], bf16)      # rows 64:128 used
            vret_T = pw.tile([128, L], bf16)      # rows 64:128 used
            vloc_nat = pw.tile([128, 8, DH + 1], bf16)
            attnT = pw.tile([128, 4, L], bf16)    # [c=512, i]

            # kT2: shipped f32 (pre-scaled by 1/sqrt(dh)); dup row halves
            nc.sync.dma_start(
                out=kT2_f[0:64, :], in_=p32_full[P32_KT:P32_KT + 64, :])
            nc.sync.dma_start(out=kT2_f[64:128, :], in_=kT2_f[0:64, :])
            nc.vector.tensor_copy(out=kT2_b[:], in_=kT2_f[:])

            # ---------------- phase A: unpack + projections ----------------
            with tc.tile_pool(name="load", bufs=1) as pl, \
                 tc.tile_pool(name="worka", bufs=2) as wa:
                qT_sb = pl.tile([128, 8, L], f32)
                kvT_sb = pl.tile([128, 8, L], bf16)
                wq_sb = pl.tile([128, 8, CPG], f32)
                wkv_sb = pl.tile([128, 8, 2 * DH], bf16)
                for kc in range(8):
                    nc.sync.dma_start(
                        out=kvT_sb[:, kc, :],
                        in_=p16_full[P16_KV:P16_KV + L,
                                     kc * 128:(kc + 1) * 128],
                        transpose=True)
                    nc.sync.dma_start(
                        out=wkv_sb[:, kc, :],
                        in_=p16_full[P16_WKV:P16_WKV + 128,
                                     kc * 128:(kc + 1) * 128],
                        transpose=True)
                    # wqT half stored flat: d-chunk kc = 64 blob rows
                    nc.sync.dma_start(
                        out=wq_sb[:, kc, :],
                        in_=e32_full[kc * 64:(kc + 1) * 64, :])

                # q [l, d] f32 -> qT_sb [d, l] via PE transposes
                for li in range(8):
                    qn = wa.tile([128, 1024], f32, tag="qn")
                    nc.sync.dma_start(
                        out=qn[:],
                        in_=p32_full[P32_Q + li * 128:P32_Q + (li + 1) * 128, :])
                    for kc in range(8):
                        tq = ps_sm.tile([128, 128], f32, tag="sm")
                        nc.tensor.transpose(
                            out=tq[:], in_=qn[:, kc * 128:(kc + 1) * 128],
                            identity=ident_f[:])
                        nc.vector.tensor_copy(
                            out=qT_sb[:, kc, li * 128:(li + 1) * 128],
                            in_=tq[:])

                # kv projection: kvpT[cc, i] (cc = 0..128 = 2*DH)
                kvp_sb = wa.tile([128, L], f32, tag="kvp")
                for ic in range(2):
                    ps = ps_av.tile([128, 512], f32, tag="av")
                    for kc in range(8):
                        nc.tensor.matmul(
                            ps[:],
                            lhsT=wkv_sb[:, kc, :],
                            rhs=kvT_sb[:, kc, ic * 512:(ic + 1) * 512],
                            start=(kc == 0), stop=(kc == 7))
                    nc.vector.tensor_copy(
                        out=kvp_sb[:, ic * 512:(ic + 1) * 512], in_=ps[:])

                # l2 norm over seq dim (free); only v rows (64:128) are used
                sqd = wa.tile([128, L], f32, tag="sqd")
                ssum = wa.tile([128, 1], f32, tag="ss")
                nc.scalar.activation(
                    out=sqd[:], in_=kvp_sb[:], func=Square, accum_out=ssum[:])
                snorm = wa.tile([128, 1], f32, tag="sn")
                nc.scalar.sqrt(out=snorm[:], in_=ssum[:])
                rn = wa.tile([128, 1], f32, tag="rn")
                nc.vector.reciprocal(out=rn[:], in_=snorm[:])

                kvn = wa.tile([128, L], f32, tag="kvn")
                nc.vector.tensor_scalar_mul(kvn[:], kvp_sb[:], rn[:, 0:1])

                # gate-folded value copies (rows 64:128)
                nc.vector.tensor_scalar_mul(
                    vloc_T[64:128, :], kvn[64:128, :], gates_sb[64:128, 1:2])
                nc.vector.tensor_scalar_mul(
                    vret_T[64:128, :], kvn[64:128, :], gates_sb[64:128, 0:1])

                # natural-layout copies: vloc (sbuf, +ones col), vret/k (dram)
                nc.vector.memset(vloc_nat[:, :, DH:DH + 1], 1.0)
                for jc in range(8):
                    tp = ps_sm.tile([128, 128], bf16, tag="sm")
                    nc.tensor.transpose(
                        out=tp[:, 0:64],
                        in_=vloc_T[64:128, jc * 128:(jc + 1) * 128],
                        identity=ident_bf[64:128, 64:128])
                    nc.vector.tensor_copy(
                        out=vloc_nat[:, jc, 0:DH], in_=tp[:, 0:64])

                    tp2 = ps_sm.tile([128, 128], bf16, tag="sm")
                    nc.tensor.transpose(
                        out=tp2[:, 0:64],
                        in_=vret_T[64:128, jc * 128:(jc + 1) * 128],
                        identity=ident_bf[64:128, 64:128])
                    vr = wa.tile([128, DH], bf16, tag="vr")
                    nc.vector.tensor_copy(out=vr[:], in_=tp2[:, 0:64])
                    nc.sync.dma_start(
                        out=v_ret_d[jc * 128:(jc + 1) * 128, :], in_=vr[:])

                    tp3 = ps_sm.tile([128, 128], bf16, tag="sm")
                    nc.tensor.transpose(
                        out=tp3[:, 0:64],
                        in_=kT2_b[0:64, jc * 128:(jc + 1) * 128],
                        identity=ident_bf[0:64, 0:64])
                    kn = wa.tile([128, DH], bf16, tag="kn")
                    nc.vector.tensor_copy(out=kn[:], in_=tp3[:, 0:64])
                    nc.sync.dma_start(
                        out=k_nat_d[jc * 128:(jc + 1) * 128, :], in_=kn[:])

                # q projection qpT[c, i]
                for cc in range(4):
                    for ic in range(2):
                        ps = ps_av.tile([128, 512], f32, tag="av")
                        for kc in range(8):
                            nc.tensor.matmul(
                                ps[:],
                                lhsT=wq_sb[:, kc, cc * 128:(cc + 1) * 128],
                                rhs=qT_sb[:, kc, ic * 512:(ic + 1) * 512],
                                start=(kc == 0), stop=(kc == 7))
                        sl = slice(ic * 512, (ic + 1) * 512)
                        nc.vector.tensor_copy(out=qpT_f[:, cc, sl], in_=ps[:])
                        nc.scalar.copy(out=qpT_b[:, cc, sl], in_=ps[:])

            # ---------------- phase B: per-head attention ----------------
            with tc.tile_pool(name="head", bufs=2) as ph:
                for h in range(HPG):
                    pb = (h % 2) * 64
                    cc = h // 2
                    qh_f = qpT_f[pb:pb + 64, cc, :]     # [64, L] f32 view
                    qh_b = qpT_b[pb:pb + 64, cc, :]     # [64, L] bf16 view

                    # --- scores S[i, j] (fp32) + argmax ---
                    idx8 = ph.tile([128, 8, 8], u32, tag="idx")
                    for qi in range(8):
                        s_ps = ps_big.tile([128, 1024], f32, tag="sbig")
                        for jh in range(2):
                            nc.tensor.matmul(
                                s_ps[:, jh * 512:(jh + 1) * 512],
                                lhsT=qh_f[:, qi * 128:(qi + 1) * 128],
                                rhs=kT2_f[pb:pb + 64, jh * 512:(jh + 1) * 512],
                                start=True, stop=True)
                        ssb = ph.tile([128, 1024], f32, tag="ssb")
                        nc.vector.tensor_copy(out=ssb[:], in_=s_ps[:])
                        m8 = ph.tile([128, 8], f32, tag="m8")
                        nc.vector.max(out=m8[:], in_=ssb[:])
                        nc.vector.max_index(
                            out=idx8[:, qi, :], in_max=m8[:], in_values=ssb[:])

                    # --- local: E = exp(S^T) ---
                    E1 = ph.tile([128, 8, 1024], bf16, tag="E1")
                    for jc in range(8):
                        st_ps = ps_big.tile([128, 1024], f32, tag="sbig")
                        for ih in range(2):
                            nc.tensor.matmul(
                                st_ps[:, ih * 512:(ih + 1) * 512],
                                lhsT=kT2_b[pb:pb + 64, jc * 128:(jc + 1) * 128],
                                rhs=qh_b[:, ih * 512:(ih + 1) * 512],
                                start=True, stop=True)
                        nc.scalar.activation(
                            out=E1[:, jc, :], in_=st_ps[:], func=Exp)

                    # --- gather retrieved k/v rows; build rkT (dup halves) ---
                    rkT = ph.tile([128, 1024], bf16, tag="rkT")
                    rv_nat = ph.tile([128, 8, DH + 1], bf16, tag="rvn")
                    nc.vector.memset(rv_nat[:, :, DH:DH + 1], 1.0)
                    for qi in range(8):
                        rk = ph.tile([128, DH], bf16, tag="rk")
                        nc.gpsimd.indirect_dma_start(
                            out=rk[:], out_offset=None,
                            in_=k_nat_d[:],
                            in_offset=bass.IndirectOffsetOnAxis(
                                ap=idx8[:, qi, 0:1], axis=0))
                        nc.gpsimd.indirect_dma_start(
                            out=rv_nat[:, qi, 0:DH], out_offset=None,
                            in_=v_ret_d[:],
                            in_offset=bass.IndirectOffsetOnAxis(
                                ap=idx8[:, qi, 0:1], axis=0))
                        tp = ps_sm.tile([128, 128], bf16, tag="sm")
                        nc.tensor.transpose(
                            out=tp[0:64, :], in_=rk[:],
                            identity=ident_bf[:, :])
                        nc.vector.tensor_copy(
                            out=rkT[0:64, qi * 128:(qi + 1) * 128],
                            in_=tp[0:64, :])
                    nc.sync.dma_start(
                        out=rkT[64:128, :], in_=rkT[0:64, :])

                    # --- retrieval: E2 = exp(S2^T) ---
                    E2 = ph.tile([128, 8, 1024], bf16, tag="E2")
                    for jc in range(8):
                        st_ps = ps_big.tile([128, 1024], f32, tag="sbig")
                        for ih in range(2):
                            nc.tensor.matmul(
                                st_ps[:, ih * 512:(ih + 1) * 512],
                                lhsT=rkT[pb:pb + 64, jc * 128:(jc + 1) * 128],
                                rhs=qh_b[:, ih * 512:(ih + 1) * 512],
                                start=True, stop=True)
                        nc.scalar.activation(
                            out=E2[:, jc, :], in_=st_ps[:], func=Exp)

                    # --- weighted sums + normalize + combine ---
                    attn_h = ph.tile([64, 1024], bf16, tag="ath")
                    for ic in range(2):
                        isl = slice(ic * 512, (ic + 1) * 512)
                        avL = ps_av.tile([65, 512], f32, tag="av")
                        avR = ps_av.tile([65, 512], f32, tag="av")
                        for jc in range(8):
                            nc.tensor.matmul(
                                avL[:], lhsT=vloc_nat[:, jc, :],
                                rhs=E1[:, jc, isl],
                                start=(jc == 0), stop=(jc == 7))
                        for jc in range(8):
                            nc.tensor.matmul(
                                avR[:], lhsT=rv_nat[:, jc, :],
                                rhs=E2[:, jc, isl],
                                start=(jc == 0), stop=(jc == 7))
                        rL = ph.tile([65, 512], f32, tag="rL")
                        rR = ph.tile([65, 512], f32, tag="rR")
                        nc.vector.reciprocal(out=rL[64:65, :], in_=avL[64:65, :])
                        nc.vector.reciprocal(out=rR[64:65, :], in_=avR[64:65, :])
                        bcL = ps_sm.tile([64, 512], f32, tag="sm")
                        bcR = ps_sm.tile([64, 512], f32, tag="sm")
                        nc.tensor.matmul(
                            bcL[:], lhsT=ones_sb[64:65, :], rhs=rL[64:65, :],
                            start=True, stop=True)
                        nc.tensor.matmul(
                            bcR[:], lhsT=ones_sb[64:65, :], rhs=rR[64:65, :],
                            start=True, stop=True)
                        bcLs = ph.tile([64, 512], f32, tag="bcLs")
                        bcRs = ph.tile([64, 512], f32, tag="bcRs")
                        nc.vector.tensor_copy(out=bcLs[:], in_=bcL[:])
                        nc.vector.tensor_copy(out=bcRs[:], in_=bcR[:])
                        bLs = ph.tile([64, 512], f32, tag="bLs")
                        bRs = ph.tile([64, 512], f32, tag="bRs")
                        nc.vector.tensor_tensor(
                            out=bLs[:], in0=avL[0:64, :], in1=bcLs[:], op=mul_op)
                        nc.vector.tensor_tensor(
                            out=bRs[:], in0=avR[0:64, :], in1=bcRs[:], op=mul_op)
                        nc.vector.tensor_add(
                            out=attn_h[:, isl], in0=bLs[:], in1=bRs[:])
                    nc.sync.dma_start(
                        out=attnT[pb:pb + 64, cc, :], in_=attn_h[:])

                # ---------------- phase C: output projection ----------------
                for mi in range(8):
                    for nh in range(2):
                        y_ps = ps_av.tile([128, 512], f32, tag="av")
                        for cc2 in range(4):
                            nc.tensor.matmul(
                                y_ps[:],
                                lhsT=attnT[:, cc2, mi * 128:(mi + 1) * 128],
                                rhs=wc_sb[:, cc2, nh * 512:(nh + 1) * 512],
                                start=(cc2 == 0), stop=(cc2 == 3))
                        y_sb = ph.tile([128, 512], bf16, tag="ysb")
                        nc.vector.tensor_copy(out=y_sb[:], in_=y_ps[:])
                        nc.sync.dma_start(
                            out=ypart[mi * 128:(mi + 1) * 128,
                                      nh * 512:(nh + 1) * 512],
                            in_=y_sb[:])

                # -------------- phase D: pair reduce + output --------------
                nc.gpsimd.collective_compute(
                    "ReduceScatter", mybir.AluOpType.add,
                    replica_groups=PAIRS,
                    ins=[ypart[:].opt()], outs=[yhalf[:].opt()])
                nc.sync.dma_start(out=y[:], in_=yhalf[:])

    import concourse.mybir as mybir2
    _split_sync_waits(nc, mybir2, max_waits=1)
    return nc


def _make_in_maps(q, kv, Wq, Wkv, Wc, bias):
    import ml_dtypes
    bf = ml_dtypes.bfloat16

    # host-side f32 keys: k = l2norm_axis1(kv @ Wkv[:64].T), pre-scaled
    kvp_k = np.einsum("bld,kd->blk", kv, Wkv[:DH], optimize=True)
    n = np.sqrt((kvp_k * kvp_k).sum(axis=1, keepdims=True))
    k = kvp_k / np.maximum(n, 1e-12)
    kT2 = 0.125 * np.ascontiguousarray(k.transpose(0, 2, 1))  # [B, 64, L]

    g = (1.0 / (1.0 + np.exp(-bias.astype(np.float64))))
    grow = np.zeros(1024, np.float32)
    grow[128:256:2] = g
    grow[129:256:2] = 1.0 - g

    blob32 = np.zeros((8, B32_R, 1024), np.float32)
    blob16 = np.zeros((8, B16_R, 1024), bf)
    wkv_b = Wkv.astype(bf)
    for bi in range(B):
        c0, c1 = 2 * bi, 2 * bi + 1
        # f32 pair content: q 0:1024 | kT2 1024:1088 | gates 1088:1089
        blob32[c0, 0:P32_SH] = q[bi][0:P32_SH]
        blob32[c1, 0:1024 - P32_SH] = q[bi][P32_SH:1024]
        blob32[c1, 1024 - P32_SH:1088 - P32_SH] = kT2[bi]
        blob32[c1, 1088 - P32_SH] = grow
        # bf16 pair content: kv 0:1024 | Wkv 1024:1152
        blob16[c0, 0:P16_SH] = kv[bi][0:P16_SH]
        blob16[c1, 0:1024 - P16_SH] = kv[bi][P16_SH:1024]
        blob16[c1, 1024 - P16_SH:P16_R - P16_SH] = wkv_b

    for hg in range(2):
        sl = slice(hg * CPG, (hg + 1) * CPG)
        e32 = np.ascontiguousarray(Wq[sl, :].T).reshape(512, 1024)
        e16 = np.ascontiguousarray(Wc[:, sl].T).astype(bf)
        for j in range(4):
            c = 2 * j + hg
            blob32[c, P32_SH:B32_R] = e32[j * E32_SH:(j + 1) * E32_SH]
            blob16[c, P16_SH:B16_R] = e16[j * E16_SH:(j + 1) * E16_SH]

    return [{"blob32": blob32[c], "blob16": blob16[c]} for c in range(8)]


def kernel(q, kv, Wq, Wkv, Wc, bias):
    from concourse.bass_utils import run_bass_kernel_spmd

    if "nc" not in _CACHE:
        _CACHE["nc"] = _build_nc()
    nc = _CACHE["nc"]

    in_maps = _make_in_maps(q, kv, Wq, Wkv, Wc, bias)
    res = run_bass_kernel_spmd(nc, in_maps, list(range(8)))
    out = np.empty((B, L, D), np.float32)
    for bi in range(B):
        out[bi, 0:512] = res.results[2 * bi]["y"].astype(np.float32)
        out[bi, 512:1024] = res.results[2 * bi + 1]["y"].astype(np.float32)
    return out


# revision 5
# speedup vs baseline: 3.0659x; 1.1441x over previous
"""KNN attention kernel for 8 Trainium2 NeuronCores.

Sharding: (batch, head-group) data parallel — core c handles batch c//2 and
heads (c%2)*8..(c%2)*8+8.  To minimize axon-tunnel traffic (the wall-clock
bottleneck), each core uploads a unique 1/8 shard of the inputs; on-device
AllGathers redistribute: pair gathers [[0,1],[2,3],..] for per-batch data and
even/odd gathers [[0,2,4,6],[1,3,5,7]] for per-head-group weight halves.

Precision split: the retrieval argmax is exquisitely sensitive to score
noise (bf16 inputs cause ~350 index flips → 4e-2 rel err), so the score
path ships in f32: q, Wq, and a host-computed kT2 = 0.125*l2norm(k).T.
The value path ships as a host-projected bf16 vT = l2norm(kv @ Wkv_v.T).T
(1 GFLOP on host, saves shipping kv/Wkv — 9 MiB of tunnel), Wc and the
output y are bf16 (~5e-3 rel err total).  Partial final projections are
pair-ReduceScattered on device so each core outputs a disjoint bf16
[512, 1024] slice of y.
"""

import sys

sys.path.insert(0, "/opt/trn_rl_repo")

import numpy as np

B, L, D, DH, H = 4, 1024, 1024, 64, 16
HPG = 8          # heads per core
CPG = HPG * DH   # channels per core (512)

# f32 blob: pair content rows [kT2 0:64 | gates 64 | q 65:1089 | pad 1089],
# eo content rows [wqT half flat 0:512]
P32_KT, P32_G, P32_Q = 0, 64, 65
P32_R, P32_SH = 1090, 545
E32_R, E32_SH = 512, 128
B32_R = P32_SH + E32_SH  # 673

# bf16 blob: pair content rows [vT 0:64], eo content rows [wcT half 0:512]
P16_V = 0
P16_R, P16_SH = 64, 32
E16_R, E16_SH = 512, 128
B16_R = P16_SH + E16_SH  # 160

_CACHE = {}


def _split_sync_waits(nc, mybir, max_waits=1):
    """This container's walrus rejects >1 sync wait per instruction; spill
    extras onto same-engine NOPs placed immediately before."""
    for fn in nc.m.functions:
        for bb in fn.blocks:
            old = list(bb.instructions)
            new_insts = []
            changed = False
            for inst in old:
                si = inst.sync_info
                if si is not None and len(si.on_wait) > max_waits:
                    waits = list(si.on_wait)
                    extra, keep = waits[:-max_waits], waits[-max_waits:]
                    k = 0
                    while extra:
                        chunk, extra = extra[:max_waits], extra[max_waits:]
                        nop = mybir.InstNoOp(
                            name=f"{inst.name}_ws{k}", ins=[], outs=[])
                        nop.engine = inst.engine
                        nop.sync_info = mybir.SyncInfo(
                            on_wait=chunk, on_update=[])
                        nc.register_instruction(nop)
                        new_insts.append(nop)
                        k += 1
                    inst.sync_info = mybir.SyncInfo(
                        on_wait=keep, on_update=list(si.on_update))
                    changed = True
                new_insts.append(inst)
            if changed:
                bb.instructions = new_insts


def _build_nc():
    import concourse.bass as bass
    import concourse.mybir as mybir
    import concourse.tile as tile
    from concourse.masks import make_identity

    f32 = mybir.dt.float32
    bf16 = mybir.dt.bfloat16
    u32 = mybir.dt.uint32
    Exp = mybir.ActivationFunctionType.Exp
    mul_op = mybir.AluOpType.mult

    nc = bass.Bass("TRN2", target_bir_lowering=False, debug=False,
                   num_devices=8)

    blob32 = nc.dram_tensor("blob32", [B32_R, 1024], f32,
                            kind="ExternalInput")
    blob16 = nc.dram_tensor("blob16", [B16_R, 1024], bf16,
                            kind="ExternalInput")
    y = nc.dram_tensor("y", [512, D], bf16, kind="ExternalOutput")

    k_nat_d = nc.dram_tensor("k_nat_d", [L, DH], bf16)
    v_ret_d = nc.dram_tensor("v_ret_d", [L, DH], bf16)

    PAIRS = [[0, 1], [2, 3], [4, 5], [6, 7]]
    EODD = [[0, 2, 4, 6], [1, 3, 5, 7]]

    with tile.TileContext(nc) as tc:
        with (
            tc.tile_pool(name="dram", bufs=1, space="DRAM") as dp,
            tc.tile_pool(name="persist", bufs=1) as pw,
            tc.tile_pool(name="psbig", bufs=2, space="PSUM") as ps_big,
            tc.tile_pool(name="psav", bufs=2, space="PSUM") as ps_av,
            tc.tile_pool(name="pssm", bufs=2, space="PSUM") as ps_sm,
        ):
            # ---------------- phase 0: gather shards ----------------
            p32_in = dp.tile([P32_SH, 1024], f32)
            p32_full = dp.tile([P32_R, 1024], f32)
            e32_in = dp.tile([E32_SH, 1024], f32)
            e32_full = dp.tile([E32_R, 1024], f32)
            p16_in = dp.tile([P16_SH, 1024], bf16)
            p16_full = dp.tile([P16_R, 1024], bf16)
            e16_in = dp.tile([E16_SH, 1024], bf16)
            e16_full = dp.tile([E16_R, 1024], bf16)
            ypart = dp.tile([L, D], bf16)
            yhalf = dp.tile([512, D], bf16)

            nc.sync.dma_start(out=p32_in[:], in_=blob32[0:P32_SH, :])
            nc.sync.dma_start(out=e32_in[:], in_=blob32[P32_SH:B32_R, :])
            nc.sync.dma_start(out=p16_in[:], in_=blob16[0:P16_SH, :])
            nc.sync.dma_start(out=e16_in[:], in_=blob16[P16_SH:B16_R, :])
            nc.gpsimd.collective_compute(
                "AllGather", mybir.AluOpType.bypass, replica_groups=PAIRS,
                ins=[p32_in[:].opt()], outs=[p32_full[:].opt()])
            nc.gpsimd.collective_compute(
                "AllGather", mybir.AluOpType.bypass, replica_groups=EODD,
                ins=[e32_in[:].opt()], outs=[e32_full[:].opt()])
            nc.gpsimd.collective_compute(
                "AllGather", mybir.AluOpType.bypass, replica_groups=PAIRS,
                ins=[p16_in[:].opt()], outs=[p16_full[:].opt()])
            nc.gpsimd.collective_compute(
                "AllGather", mybir.AluOpType.bypass, replica_groups=EODD,
                ins=[e16_in[:].opt()], outs=[e16_full[:].opt()])

            ident_bf = pw.tile([128, 128], bf16)
            make_identity(nc, ident_bf[:])
            ident_f = pw.tile([128, 128], f32)
            make_identity(nc, ident_f[:])
            ones_sb = pw.tile([128, 64], f32)
            nc.vector.memset(ones_sb[:], 1.0)

            # gates [128, 2] f32 <- one interleaved f32 row
            gates_sb = pw.tile([128, 2], f32)
            nc.sync.dma_start(
                out=gates_sb[:], in_=p32_full[P32_G:P32_G + 1, 0:256])

            # wc_sb[:, cc, :] = wcT rows (hg half, device-invariant layout)
            wc_sb = pw.tile([128, 4, D], bf16)
            for cc in range(4):
                nc.sync.dma_start(
                    out=wc_sb[:, cc, :],
                    in_=e16_full[cc * 128:(cc + 1) * 128, :])

            qpT_f = pw.tile([128, 4, L], f32)     # [c=512, i] c=cc*128+p
            qpT_b = pw.tile([128, 4, L], bf16)
            kT2_f = pw.tile([128, L], f32)        # rows 0:64 kT, 64:128 dup
            kT2_b = pw.tile([128, L], bf16)
            vloc_T = pw.tile([128, L], bf16)      # rows 64:128 used
            vret_T = pw.tile([128, L], bf16)      # rows 64:128 used
            vloc_nat = pw.tile([128, 8, DH + 1], bf16)
            attnT = pw.tile([128, 4, L], bf16)    # [c=512, i]

            # kT2: shipped f32 (pre-scaled by 1/sqrt(dh)); dup row halves
            nc.sync.dma_start(
                out=kT2_f[0:64, :], in_=p32_full[P32_KT:P32_KT + 64, :])
            nc.sync.dma_start(out=kT2_f[64:128, :], in_=kT2_f[0:64, :])
            nc.vector.tensor_copy(out=kT2_b[:], in_=kT2_f[:])

            # vT: shipped bf16, load to partitions 64:128; fold gates
            vT_sb = pw.tile([128, L], bf16)
            nc.sync.dma_start(
                out=vT_sb[64:128, :], in_=p16_full[P16_V:P16_V + 64, :])
            nc.vector.tensor_scalar_mul(
                vloc_T[64:128, :], vT_sb[64:128, :], gates_sb[64:128, 1:2])
            nc.vector.tensor_scalar_mul(
                vret_T[64:128, :], vT_sb[64:128, :], gates_sb[64:128, 0:1])

            # ---------------- phase A: unpack + projections ----------------
            with tc.tile_pool(name="load", bufs=1) as pl, \
                 tc.tile_pool(name="worka", bufs=2) as wa:
                qT_sb = pl.tile([128, 8, L], f32)
                wq_sb = pl.tile([128, 8, CPG], f32)
                for kc in range(8):
                    # wqT half stored flat: d-chunk kc = 64 blob rows
                    nc.sync.dma_start(
                        out=wq_sb[:, kc, :],
                        in_=e32_full[kc * 64:(kc + 1) * 64, :])

                # q [l, d] f32 -> qT_sb [d, l] via PE transposes
                for li in range(8):
                    qn = wa.tile([128, 1024], f32, tag="qn")
                    nc.sync.dma_start(
                        out=qn[:],
                        in_=p32_full[P32_Q + li * 128:P32_Q + (li + 1) * 128, :])
                    for kc in range(8):
                        tq = ps_sm.tile([128, 128], f32, tag="sm")
                        nc.tensor.transpose(
                            out=tq[:], in_=qn[:, kc * 128:(kc + 1) * 128],
                            identity=ident_f[:])
                        nc.vector.tensor_copy(
                            out=qT_sb[:, kc, li * 128:(li + 1) * 128],
                            in_=tq[:])

                # natural-layout copies: vloc (sbuf, +ones col), vret/k (dram)
                nc.vector.memset(vloc_nat[:, :, DH:DH + 1], 1.0)
                for jc in range(8):
                    tp = ps_sm.tile([128, 128], bf16, tag="sm")
                    nc.tensor.transpose(
                        out=tp[:, 0:64],
                        in_=vloc_T[64:128, jc * 128:(jc + 1) * 128],
                        identity=ident_bf[64:128, 64:128])
                    nc.vector.tensor_copy(
                        out=vloc_nat[:, jc, 0:DH], in_=tp[:, 0:64])

                    tp2 = ps_sm.tile([128, 128], bf16, tag="sm")
                    nc.tensor.transpose(
                        out=tp2[:, 0:64],
                        in_=vret_T[64:128, jc * 128:(jc + 1) * 128],
                        identity=ident_bf[64:128, 64:128])
                    vr = wa.tile([128, DH], bf16, tag="vr")
                    nc.vector.tensor_copy(out=vr[:], in_=tp2[:, 0:64])
                    nc.sync.dma_start(
                        out=v_ret_d[jc * 128:(jc + 1) * 128, :], in_=vr[:])

                    tp3 = ps_sm.tile([128, 128], bf16, tag="sm")
                    nc.tensor.transpose(
                        out=tp3[:, 0:64],
                        in_=kT2_b[0:64, jc * 128:(jc + 1) * 128],
                        identity=ident_bf[0:64, 0:64])
                    kn = wa.tile([128, DH], bf16, tag="kn")
                    nc.vector.tensor_copy(out=kn[:], in_=tp3[:, 0:64])
                    nc.sync.dma_start(
                        out=k_nat_d[jc * 128:(jc + 1) * 128, :], in_=kn[:])

                # q projection qpT[c, i]
                for cc in range(4):
                    for ic in range(2):
                        ps = ps_av.tile([128, 512], f32, tag="av")
                        for kc in range(8):
                            nc.tensor.matmul(
                                ps[:],
                                lhsT=wq_sb[:, kc, cc * 128:(cc + 1) * 128],
                                rhs=qT_sb[:, kc, ic * 512:(ic + 1) * 512],
                                start=(kc == 0), stop=(kc == 7))
                        sl = slice(ic * 512, (ic + 1) * 512)
                        nc.vector.tensor_copy(out=qpT_f[:, cc, sl], in_=ps[:])
                        nc.scalar.copy(out=qpT_b[:, cc, sl], in_=ps[:])

            # ---------------- phase B: per-head attention ----------------
            with tc.tile_pool(name="head", bufs=2) as ph:
                for h in range(HPG):
                    pb = (h % 2) * 64
                    cc = h // 2
                    qh_f = qpT_f[pb:pb + 64, cc, :]     # [64, L] f32 view
                    qh_b = qpT_b[pb:pb + 64, cc, :]     # [64, L] bf16 view

                    # --- scores S[i, j] (fp32) + argmax ---
                    idx8 = ph.tile([128, 8, 8], u32, tag="idx")
                    for qi in range(8):
                        s_ps = ps_big.tile([128, 1024], f32, tag="sbig")
                        for jh in range(2):
                            nc.tensor.matmul(
                                s_ps[:, jh * 512:(jh + 1) * 512],
                                lhsT=qh_f[:, qi * 128:(qi + 1) * 128],
                                rhs=kT2_f[pb:pb + 64, jh * 512:(jh + 1) * 512],
                                start=True, stop=True)
                        ssb = ph.tile([128, 1024], f32, tag="ssb")
                        nc.vector.tensor_copy(out=ssb[:], in_=s_ps[:])
                        m8 = ph.tile([128, 8], f32, tag="m8")
                        nc.vector.max(out=m8[:], in_=ssb[:])
                        nc.vector.max_index(
                            out=idx8[:, qi, :], in_max=m8[:], in_values=ssb[:])

                    # --- local: E = exp(S^T) ---
                    E1 = ph.tile([128, 8, 1024], bf16, tag="E1")
                    for jc in range(8):
                        st_ps = ps_big.tile([128, 1024], f32, tag="sbig")
                        for ih in range(2):
                            nc.tensor.matmul(
                                st_ps[:, ih * 512:(ih + 1) * 512],
                                lhsT=kT2_b[pb:pb + 64, jc * 128:(jc + 1) * 128],
                                rhs=qh_b[:, ih * 512:(ih + 1) * 512],
                                start=True, stop=True)
                        nc.scalar.activation(
                            out=E1[:, jc, :], in_=st_ps[:], func=Exp)

                    # --- gather retrieved k/v rows; build rkT (dup halves) ---
                    rkT = ph.tile([128, 1024], bf16, tag="rkT")
                    rv_nat = ph.tile([128, 8, DH + 1], bf16, tag="rvn")
                    nc.vector.memset(rv_nat[:, :, DH:DH + 1], 1.0)
                    for qi in range(8):
                        rk = ph.tile([128, DH], bf16, tag="rk")
                        nc.gpsimd.indirect_dma_start(
                            out=rk[:], out_offset=None,
                            in_=k_nat_d[:],
                            in_offset=bass.IndirectOffsetOnAxis(
                                ap=idx8[:, qi, 0:1], axis=0))
                        nc.gpsimd.indirect_dma_start(
                            out=rv_nat[:, qi, 0:DH], out_offset=None,
                            in_=v_ret_d[:],
                            in_offset=bass.IndirectOffsetOnAxis(
                                ap=idx8[:, qi, 0:1], axis=0))
                        tp = ps_sm.tile([128, 128], bf16, tag="sm")
                        nc.tensor.transpose(
                            out=tp[0:64, :], in_=rk[:],
                            identity=ident_bf[:, :])
                        nc.vector.tensor_copy(
                            out=rkT[0:64, qi * 128:(qi + 1) * 128],
                            in_=tp[0:64, :])
                    nc.sync.dma_start(
                        out=rkT[64:128, :], in_=rkT[0:64, :])

                    # --- retrieval: E2 = exp(S2^T) ---
                    E2 = ph.tile([128, 8, 1024], bf16, tag="E2")
                    for jc in range(8):
                        st_ps = ps_big.tile([128, 1024], f32, tag="sbig")
                        for ih in range(2):
                            nc.tensor.matmul(
                                st_ps[:, ih * 512:(ih + 1) * 512],
                                lhsT=rkT[pb:pb + 64, jc * 128:(jc + 1) * 128],
                                rhs=qh_b[:, ih * 512:(ih + 1) * 512],
                                start=True, stop=True)
                        nc.scalar.activation(
                            out=E2[:, jc, :], in_=st_ps[:], func=Exp)

                    # --- weighted sums + normalize + combine ---
                    attn_h = ph.tile([64, 1024], bf16, tag="ath")
                    for ic in range(2):
                        isl = slice(ic * 512, (ic + 1) * 512)
                        avL = ps_av.tile([65, 512], f32, tag="av")
                        avR = ps_av.tile([65, 512], f32, tag="av")
                        for jc in range(8):
                            nc.tensor.matmul(
                                avL[:], lhsT=vloc_nat[:, jc, :],
                                rhs=E1[:, jc, isl],
                                start=(jc == 0), stop=(jc == 7))
                        for jc in range(8):
                            nc.tensor.matmul(
                                avR[:], lhsT=rv_nat[:, jc, :],
                                rhs=E2[:, jc, isl],
                                start=(jc == 0), stop=(jc == 7))
                        rL = ph.tile([65, 512], f32, tag="rL")
                        rR = ph.tile([65, 512], f32, tag="rR")
                        nc.vector.reciprocal(out=rL[64:65, :], in_=avL[64:65, :])
                        nc.vector.reciprocal(out=rR[64:65, :], in_=avR[64:65, :])
                        bcL = ps_sm.tile([64, 512], f32, tag="sm")
                        bcR = ps_sm.tile([64, 512], f32, tag="sm")
                        nc.tensor.matmul(
                            bcL[:], lhsT=ones_sb[64:65, :], rhs=rL[64:65, :],
                            start=True, stop=True)
                        nc.tensor.matmul(
                            bcR[:], lhsT=ones_sb[64:65, :], rhs=rR[64:65, :],
                            start=True, stop=True)
                        bcLs = ph.tile([64, 512], f32, tag="bcLs")
                        bcRs = ph.tile([64, 512], f32, tag="bcRs")
                        nc.vector.tensor_copy(out=bcLs[:], in_=bcL[:])
                        nc.vector.tensor_copy(out=bcRs[:], in_=bcR[:])
                        bLs = ph.tile([64, 512], f32, tag="bLs")
                        bRs = ph.tile([64, 512], f32, tag="bRs")
                        nc.vector.tensor_tensor(
                            out=bLs[:], in0=avL[0:64, :], in1=bcLs[:], op=mul_op)
                        nc.vector.tensor_tensor(
                            out=bRs[:], in0=avR[0:64, :], in1=bcRs[:], op=mul_op)
                        nc.vector.tensor_add(
                            out=attn_h[:, isl], in0=bLs[:], in1=bRs[:])
                    nc.sync.dma_start(
                        out=attnT[pb:pb + 64, cc, :], in_=attn_h[:])

                # ---------------- phase C: output projection ----------------
                for mi in range(8):
                    for nh in range(2):
                        y_ps = ps_av.tile([128, 512], f32, tag="av")
                        for cc2 in range(4):
                            nc.tensor.matmul(
                                y_ps[:],
                                lhsT=attnT[:, cc2, mi * 128:(mi + 1) * 128],
                                rhs=wc_sb[:, cc2, nh * 512:(nh + 1) * 512],
                                start=(cc2 == 0), stop=(cc2 == 3))
                        y_sb = ph.tile([128, 512], bf16, tag="ysb")
                        nc.vector.tensor_copy(out=y_sb[:], in_=y_ps[:])
                        nc.sync.dma_start(
                            out=ypart[mi * 128:(mi + 1) * 128,
                                      nh * 512:(nh + 1) * 512],
                            in_=y_sb[:])

                # -------------- phase D: pair reduce + output --------------
                nc.gpsimd.collective_compute(
                    "ReduceScatter", mybir.AluOpType.add,
                    replica_groups=PAIRS,
                    ins=[ypart[:].opt()], outs=[yhalf[:].opt()])
                nc.sync.dma_start(out=y[:], in_=yhalf[:])

    import concourse.mybir as mybir2
    _split_sync_waits(nc, mybir2, max_waits=1)
    return nc


def _make_in_maps(q, kv, Wq, Wkv, Wc, bias):
    import ml_dtypes
    bf = ml_dtypes.bfloat16

    # host-side f32 kv projection + l2 norm over seq:
    # k pre-scaled by 1/sqrt(dh), v shipped bf16
    kvp = np.einsum("bld,kd->blk", kv, Wkv, optimize=True)  # [B, L, 128]
    n = np.sqrt((kvp * kvp).sum(axis=1, keepdims=True))
    kvn = kvp / np.maximum(n, 1e-12)
    kvnT = kvn.transpose(0, 2, 1)                           # [B, 128, L]
    kT2 = 0.125 * np.ascontiguousarray(kvnT[:, :DH])        # [B, 64, L]
    vT = np.ascontiguousarray(kvnT[:, DH:]).astype(bf)      # [B, 64, L]

    g = 1.0 / (1.0 + np.exp(-bias.astype(np.float64)))
    grow = np.zeros(1024, np.float32)
    grow[128:256:2] = g
    grow[129:256:2] = 1.0 - g

    blob32 = np.zeros((8, B32_R, 1024), np.float32)
    blob16 = np.zeros((8, B16_R, 1024), bf)
    for bi in range(B):
        c0, c1 = 2 * bi, 2 * bi + 1
        # f32 pair content: kT2 0:64 | gates 64 | q 65:1089 | pad
        blob32[c0, 0:64] = kT2[bi]
        blob32[c0, 64] = grow
        blob32[c0, 65:P32_SH] = q[bi][0:P32_SH - 65]
        blob32[c1, 0:1089 - P32_SH] = q[bi][P32_SH - 65:1024]
        # bf16 pair content: vT 0:64
        blob16[c0, 0:P16_SH] = vT[bi][0:P16_SH]
        blob16[c1, 0:P16_SH] = vT[bi][P16_SH:64]

    for hg in range(2):
        sl = slice(hg * CPG, (hg + 1) * CPG)
        e32 = np.ascontiguousarray(Wq[sl, :].T).reshape(512, 1024)
        e16 = np.ascontiguousarray(Wc[:, sl].T).astype(bf)
        for j in range(4):
            c = 2 * j + hg
            blob32[c, P32_SH:B32_R] = e32[j * E32_SH:(j + 1) * E32_SH]
            blob16[c, P16_SH:B16_R] = e16[j * E16_SH:(j + 1) * E16_SH]

    return [{"blob32": blob32[c], "blob16": blob16[c]} for c in range(8)]


def kernel(q, kv, Wq, Wkv, Wc, bias):
    from concourse.bass_utils import run_bass_kernel_spmd

    if "nc" not in _CACHE:
        _CACHE["nc"] = _build_nc()
    nc = _CACHE["nc"]

    in_maps = _make_in_maps(q, kv, Wq, Wkv, Wc, bias)
    res = run_bass_kernel_spmd(nc, in_maps, list(range(8)))
    out = np.empty((B, L, D), np.float32)
    for bi in range(B):
        out[bi, 0:512] = res.results[2 * bi]["y"].astype(np.float32)
        out[bi, 512:1024] = res.results[2 * bi + 1]["y"].astype(np.float32)
    return out


# revision 6
# speedup vs baseline: 3.5749x; 1.1660x over previous
"""KNN attention kernel for 8 Trainium2 NeuronCores.

Sharding: (batch, head-group) data parallel — core c handles batch c//2 and
heads (c%2)*8..(c%2)*8+8.  To minimize axon-tunnel traffic (the wall-clock
bottleneck), each core uploads a unique 1/8 shard of the inputs; on-device
AllGathers redistribute: pair gathers [[0,1],[2,3],..] for per-batch data and
an even/odd gather [[0,2,4,6],[1,3,5,7]] for per-head-group weight halves.

Precision scheme: the retrieval argmax is exquisitely sensitive to score
noise (bf16 inputs cause ~350 index flips → 4e-2 rel err; int16 fixed-point
causes only 4).  q and Wq ship as int16 fixed-point (converted to f32 on
device; the quantization scales fold into the host-shipped keys
kT2 = 0.125*sq*sw*l2norm(k).T, since argmax is scale-invariant and the
softmax scores see the product fold — no runtime scales needed on device).
The value path ships as a host-projected bf16 vT = l2norm(kv @ Wkv_v.T).T
(1 GFLOP on host, saves shipping kv/Wkv), Wc and the output y are bf16.
Partial final projections are pair-ReduceScattered on device so each core
outputs a disjoint bf16 [512, 1024] slice of y.
"""

import sys

sys.path.insert(0, "/opt/trn_rl_repo")

import numpy as np

B, L, D, DH, H = 4, 1024, 1024, 64, 16
HPG = 8          # heads per core
CPG = HPG * DH   # channels per core (512)

# f32 blob: pair content rows [kT2 0:64 | gates 64 | pad 65]
P32_KT, P32_G = 0, 64
P32_R, P32_SH = 66, 33
B32_R = 33

# bf16-typed blob: pair content rows [vT 0:64 | q int16 bits 64:1088],
# eo content rows [wcT half 0:512 | wqT-half int16 bits flat 512:1024]
P16_V, P16_Q = 0, 64
P16_R, P16_SH = 1088, 544
E16_WC, E16_WQ = 0, 512
E16_R, E16_SH = 1024, 256
B16_R = P16_SH + E16_SH  # 800

QSCALE = 32766.0

_CACHE = {}


def _split_sync_waits(nc, mybir, max_waits=1):
    """This container's walrus rejects >1 sync wait per instruction; spill
    extras onto same-engine NOPs placed immediately before."""
    for fn in nc.m.functions:
        for bb in fn.blocks:
            old = list(bb.instructions)
            new_insts = []
            changed = False
            for inst in old:
                si = inst.sync_info
                if si is not None and len(si.on_wait) > max_waits:
                    waits = list(si.on_wait)
                    extra, keep = waits[:-max_waits], waits[-max_waits:]
                    k = 0
                    while extra:
                        chunk, extra = extra[:max_waits], extra[max_waits:]
                        nop = mybir.InstNoOp(
                            name=f"{inst.name}_ws{k}", ins=[], outs=[])
                        nop.engine = inst.engine
                        nop.sync_info = mybir.SyncInfo(
                            on_wait=chunk, on_update=[])
                        nc.register_instruction(nop)
                        new_insts.append(nop)
                        k += 1
                    inst.sync_info = mybir.SyncInfo(
                        on_wait=keep, on_update=list(si.on_update))
                    changed = True
                new_insts.append(inst)
            if changed:
                bb.instructions = new_insts


def _build_nc():
    import concourse.bass as bass
    import concourse.mybir as mybir
    import concourse.tile as tile
    from concourse.masks import make_identity

    f32 = mybir.dt.float32
    bf16 = mybir.dt.bfloat16
    i16 = mybir.dt.int16
    u32 = mybir.dt.uint32
    Exp = mybir.ActivationFunctionType.Exp
    mul_op = mybir.AluOpType.mult

    nc = bass.Bass("TRN2", target_bir_lowering=False, debug=False,
                   num_devices=8)

    blob32 = nc.dram_tensor("blob32", [B32_R, 1024], f32,
                            kind="ExternalInput")
    blob16 = nc.dram_tensor("blob16", [B16_R, 1024], bf16,
                            kind="ExternalInput")
    y = nc.dram_tensor("y", [512, D], bf16, kind="ExternalOutput")

    k_nat_d = nc.dram_tensor("k_nat_d", [L, DH], bf16)
    v_ret_d = nc.dram_tensor("v_ret_d", [L, DH], bf16)

    PAIRS = [[0, 1], [2, 3], [4, 5], [6, 7]]
    EODD = [[0, 2, 4, 6], [1, 3, 5, 7]]

    with tile.TileContext(nc) as tc:
        with (
            tc.tile_pool(name="dram", bufs=1, space="DRAM") as dp,
            tc.tile_pool(name="persist", bufs=1) as pw,
            tc.tile_pool(name="psbig", bufs=2, space="PSUM") as ps_big,
            tc.tile_pool(name="psav", bufs=2, space="PSUM") as ps_av,
            tc.tile_pool(name="pssm", bufs=2, space="PSUM") as ps_sm,
        ):
            # ---------------- phase 0: gather shards ----------------
            p32_in = dp.tile([P32_SH, 1024], f32)
            p32_full = dp.tile([P32_R, 1024], f32)
            p16_in = dp.tile([P16_SH, 1024], bf16)
            p16_full = dp.tile([P16_R, 1024], bf16)
            e16_in = dp.tile([E16_SH, 1024], bf16)
            e16_full = dp.tile([E16_R, 1024], bf16)
            ypart = dp.tile([L, D], bf16)
            yhalf = dp.tile([512, D], bf16)

            nc.sync.dma_start(out=p32_in[:], in_=blob32[0:P32_SH, :])
            nc.sync.dma_start(out=p16_in[:], in_=blob16[0:P16_SH, :])
            nc.sync.dma_start(out=e16_in[:], in_=blob16[P16_SH:B16_R, :])
            nc.gpsimd.collective_compute(
                "AllGather", mybir.AluOpType.bypass, replica_groups=PAIRS,
                ins=[p32_in[:].opt()], outs=[p32_full[:].opt()])
            nc.gpsimd.collective_compute(
                "AllGather", mybir.AluOpType.bypass, replica_groups=PAIRS,
                ins=[p16_in[:].opt()], outs=[p16_full[:].opt()])
            nc.gpsimd.collective_compute(
                "AllGather", mybir.AluOpType.bypass, replica_groups=EODD,
                ins=[e16_in[:].opt()], outs=[e16_full[:].opt()])

            ident_bf = pw.tile([128, 128], bf16)
            make_identity(nc, ident_bf[:])
            ones_sb = pw.tile([128, 64], f32)
            nc.vector.memset(ones_sb[:], 1.0)

            # gates [128, 2] f32 <- one interleaved f32 row
            gates_sb = pw.tile([128, 2], f32)
            nc.sync.dma_start(
                out=gates_sb[:], in_=p32_full[P32_G:P32_G + 1, 0:256])

            # wc_sb[:, cc, :] = wcT rows (hg half, device-invariant layout)
            wc_sb = pw.tile([128, 4, D], bf16)
            for cc in range(4):
                nc.sync.dma_start(
                    out=wc_sb[:, cc, :],
                    in_=e16_full[E16_WC + cc * 128:E16_WC + (cc + 1) * 128, :])

            qpT_f = pw.tile([128, 4, L], f32)     # [c=512, i] c=cc*128+p
            qpT_b = pw.tile([128, 4, L], bf16)
            kT2_f = pw.tile([128, L], f32)        # rows 0:64 kT, 64:128 dup
            kT2_b = pw.tile([128, L], bf16)
            vloc_T = pw.tile([128, L], bf16)      # rows 64:128 used
            vret_T = pw.tile([128, L], bf16)      # rows 64:128 used
            vloc_nat = pw.tile([128, 8, DH + 1], bf16)
            attnT = pw.tile([128, 4, L], bf16)    # [c=512, i]

            # kT2: shipped f32, pre-scaled by sq*sw/sqrt(dh); dup row halves
            nc.sync.dma_start(
                out=kT2_f[0:64, :], in_=p32_full[P32_KT:P32_KT + 64, :])
            nc.sync.dma_start(out=kT2_f[64:128, :], in_=kT2_f[0:64, :])
            nc.vector.tensor_copy(out=kT2_b[:], in_=kT2_f[:])

            # vT: shipped bf16, load to partitions 64:128; fold gates
            vT_sb = pw.tile([128, L], bf16)
            nc.sync.dma_start(
                out=vT_sb[64:128, :], in_=p16_full[P16_V:P16_V + 64, :])
            nc.vector.tensor_scalar_mul(
                vloc_T[64:128, :], vT_sb[64:128, :], gates_sb[64:128, 1:2])
            nc.vector.tensor_scalar_mul(
                vret_T[64:128, :], vT_sb[64:128, :], gates_sb[64:128, 0:1])

            # ---------------- phase A: unpack + projections ----------------
            with tc.tile_pool(name="load", bufs=1) as pl, \
                 tc.tile_pool(name="worka", bufs=2) as wa:
                # q int16 bits: transpose-load then convert to f32
                qT_bits = pl.tile([128, 8, L], bf16)
                qT_sb = pl.tile([128, 8, L], f32)
                wq_bits = pl.tile([128, 8, CPG], bf16)
                wq_sb = pl.tile([128, 8, CPG], f32)
                for kc in range(8):
                    nc.sync.dma_start(
                        out=qT_bits[:, kc, :],
                        in_=p16_full[P16_Q:P16_Q + L, kc * 128:(kc + 1) * 128],
                        transpose=True)
                    # wqT half int16 stored flat: d-chunk kc = 64 blob rows
                    nc.sync.dma_start(
                        out=wq_bits[:, kc, :],
                        in_=e16_full[E16_WQ + kc * 64:E16_WQ + (kc + 1) * 64, :])
                nc.vector.tensor_copy(
                    out=qT_sb[:], in_=qT_bits[:].bitcast(i16))
                nc.vector.tensor_copy(
                    out=wq_sb[:], in_=wq_bits[:].bitcast(i16))

                # natural-layout copies: vloc (sbuf, +ones col), vret/k (dram)
                nc.vector.memset(vloc_nat[:, :, DH:DH + 1], 1.0)
                for jc in range(8):
                    tp = ps_sm.tile([128, 128], bf16, tag="sm")
                    nc.tensor.transpose(
                        out=tp[:, 0:64],
                        in_=vloc_T[64:128, jc * 128:(jc + 1) * 128],
                        identity=ident_bf[64:128, 64:128])
                    nc.vector.tensor_copy(
                        out=vloc_nat[:, jc, 0:DH], in_=tp[:, 0:64])

                    tp2 = ps_sm.tile([128, 128], bf16, tag="sm")
                    nc.tensor.transpose(
                        out=tp2[:, 0:64],
                        in_=vret_T[64:128, jc * 128:(jc + 1) * 128],
                        identity=ident_bf[64:128, 64:128])
                    vr = wa.tile([128, DH], bf16, tag="vr")
                    nc.vector.tensor_copy(out=vr[:], in_=tp2[:, 0:64])
                    nc.sync.dma_start(
                        out=v_ret_d[jc * 128:(jc + 1) * 128, :], in_=vr[:])

                    tp3 = ps_sm.tile([128, 128], bf16, tag="sm")
                    nc.tensor.transpose(
                        out=tp3[:, 0:64],
                        in_=kT2_b[0:64, jc * 128:(jc + 1) * 128],
                        identity=ident_bf[0:64, 0:64])
                    kn = wa.tile([128, DH], bf16, tag="kn")
                    nc.vector.tensor_copy(out=kn[:], in_=tp3[:, 0:64])
                    nc.sync.dma_start(
                        out=k_nat_d[jc * 128:(jc + 1) * 128, :], in_=kn[:])

                # q projection qpT[c, i] (values scaled by 1/(sq*sw))
                for cc in range(4):
                    for ic in range(2):
                        ps = ps_av.tile([128, 512], f32, tag="av")
                        for kc in range(8):
                            nc.tensor.matmul(
                                ps[:],
                                lhsT=wq_sb[:, kc, cc * 128:(cc + 1) * 128],
                                rhs=qT_sb[:, kc, ic * 512:(ic + 1) * 512],
                                start=(kc == 0), stop=(kc == 7))
                        sl = slice(ic * 512, (ic + 1) * 512)
                        nc.vector.tensor_copy(out=qpT_f[:, cc, sl], in_=ps[:])
                        nc.scalar.copy(out=qpT_b[:, cc, sl], in_=ps[:])

            # ---------------- phase B: per-head attention ----------------
            with tc.tile_pool(name="head", bufs=2) as ph:
                for h in range(HPG):
                    pb = (h % 2) * 64
                    cc = h // 2
                    qh_f = qpT_f[pb:pb + 64, cc, :]     # [64, L] f32 view
                    qh_b = qpT_b[pb:pb + 64, cc, :]     # [64, L] bf16 view

                    # --- scores S[i, j] (fp32) + argmax ---
                    idx8 = ph.tile([128, 8, 8], u32, tag="idx")
                    for qi in range(8):
                        s_ps = ps_big.tile([128, 1024], f32, tag="sbig")
                        for jh in range(2):
                            nc.tensor.matmul(
                                s_ps[:, jh * 512:(jh + 1) * 512],
                                lhsT=qh_f[:, qi * 128:(qi + 1) * 128],
                                rhs=kT2_f[pb:pb + 64, jh * 512:(jh + 1) * 512],
                                start=True, stop=True)
                        ssb = ph.tile([128, 1024], f32, tag="ssb")
                        nc.vector.tensor_copy(out=ssb[:], in_=s_ps[:])
                        m8 = ph.tile([128, 8], f32, tag="m8")
                        nc.vector.max(out=m8[:], in_=ssb[:])
                        nc.vector.max_index(
                            out=idx8[:, qi, :], in_max=m8[:], in_values=ssb[:])

                    # --- local: E = exp(S^T) ---
                    E1 = ph.tile([128, 8, 1024], bf16, tag="E1")
                    for jc in range(8):
                        st_ps = ps_big.tile([128, 1024], f32, tag="sbig")
                        for ih in range(2):
                            nc.tensor.matmul(
                                st_ps[:, ih * 512:(ih + 1) * 512],
                                lhsT=kT2_b[pb:pb + 64, jc * 128:(jc + 1) * 128],
                                rhs=qh_b[:, ih * 512:(ih + 1) * 512],
                                start=True, stop=True)
                        nc.scalar.activation(
                            out=E1[:, jc, :], in_=st_ps[:], func=Exp)

                    # --- gather retrieved k/v rows; build rkT (dup halves) ---
                    rkT = ph.tile([128, 1024], bf16, tag="rkT")
                    rv_nat = ph.tile([128, 8, DH + 1], bf16, tag="rvn")
                    nc.vector.memset(rv_nat[:, :, DH:DH + 1], 1.0)
                    for qi in range(8):
                        rk = ph.tile([128, DH], bf16, tag="rk")
                        nc.gpsimd.indirect_dma_start(
                            out=rk[:], out_offset=None,
                            in_=k_nat_d[:],
                            in_offset=bass.IndirectOffsetOnAxis(
                                ap=idx8[:, qi, 0:1], axis=0))
                        nc.gpsimd.indirect_dma_start(
                            out=rv_nat[:, qi, 0:DH], out_offset=None,
                            in_=v_ret_d[:],
                            in_offset=bass.IndirectOffsetOnAxis(
                                ap=idx8[:, qi, 0:1], axis=0))
                        tp = ps_sm.tile([128, 128], bf16, tag="sm")
                        nc.tensor.transpose(
                            out=tp[0:64, :], in_=rk[:],
                            identity=ident_bf[:, :])
                        nc.vector.tensor_copy(
                            out=rkT[0:64, qi * 128:(qi + 1) * 128],
                            in_=tp[0:64, :])
                    nc.sync.dma_start(
                        out=rkT[64:128, :], in_=rkT[0:64, :])

                    # --- retrieval: E2 = exp(S2^T) ---
                    E2 = ph.tile([128, 8, 1024], bf16, tag="E2")
                    for jc in range(8):
                        st_ps = ps_big.tile([128, 1024], f32, tag="sbig")
                        for ih in range(2):
                            nc.tensor.matmul(
                                st_ps[:, ih * 512:(ih + 1) * 512],
                                lhsT=rkT[pb:pb + 64, jc * 128:(jc + 1) * 128],
                                rhs=qh_b[:, ih * 512:(ih + 1) * 512],
                                start=True, stop=True)
                        nc.scalar.activation(
                            out=E2[:, jc, :], in_=st_ps[:], func=Exp)

                    # --- weighted sums + normalize + combine ---
                    attn_h = ph.tile([64, 1024], bf16, tag="ath")
                    for ic in range(2):
                        isl = slice(ic * 512, (ic + 1) * 512)
                        avL = ps_av.tile([65, 512], f32, tag="av")
                        avR = ps_av.tile([65, 512], f32, tag="av")
                        for jc in range(8):
                            nc.tensor.matmul(
                                avL[:], lhsT=vloc_nat[:, jc, :],
                                rhs=E1[:, jc, isl],
                                start=(jc == 0), stop=(jc == 7))
                        for jc in range(8):
                            nc.tensor.matmul(
                                avR[:], lhsT=rv_nat[:, jc, :],
                                rhs=E2[:, jc, isl],
                                start=(jc == 0), stop=(jc == 7))
                        rL = ph.tile([65, 512], f32, tag="rL")
                        rR = ph.tile([65, 512], f32, tag="rR")
                        nc.vector.reciprocal(out=rL[64:65, :], in_=avL[64:65, :])
                        nc.vector.reciprocal(out=rR[64:65, :], in_=avR[64:65, :])
                        bcL = ps_sm.tile([64, 512], f32, tag="sm")
                        bcR = ps_sm.tile([64, 512], f32, tag="sm")
                        nc.tensor.matmul(
                            bcL[:], lhsT=ones_sb[64:65, :], rhs=rL[64:65, :],
                            start=True, stop=True)
                        nc.tensor.matmul(
                            bcR[:], lhsT=ones_sb[64:65, :], rhs=rR[64:65, :],
                            start=True, stop=True)
                        bcLs = ph.tile([64, 512], f32, tag="bcLs")
                        bcRs = ph.tile([64, 512], f32, tag="bcRs")
                        nc.vector.tensor_copy(out=bcLs[:], in_=bcL[:])
                        nc.vector.tensor_copy(out=bcRs[:], in_=bcR[:])
                        bLs = ph.tile([64, 512], f32, tag="bLs")
                        bRs = ph.tile([64, 512], f32, tag="bRs")
                        nc.vector.tensor_tensor(
                            out=bLs[:], in0=avL[0:64, :], in1=bcLs[:], op=mul_op)
                        nc.vector.tensor_tensor(
                            out=bRs[:], in0=avR[0:64, :], in1=bcRs[:], op=mul_op)
                        nc.vector.tensor_add(
                            out=attn_h[:, isl], in0=bLs[:], in1=bRs[:])
                    nc.sync.dma_start(
                        out=attnT[pb:pb + 64, cc, :], in_=attn_h[:])

                # ---------------- phase C: output projection ----------------
                for mi in range(8):
                    for nh in range(2):
                        y_ps = ps_av.tile([128, 512], f32, tag="av")
                        for cc2 in range(4):
                            nc.tensor.matmul(
                                y_ps[:],
                                lhsT=attnT[:, cc2, mi * 128:(mi + 1) * 128],
                                rhs=wc_sb[:, cc2, nh * 512:(nh + 1) * 512],
                                start=(cc2 == 0), stop=(cc2 == 3))
                        y_sb = ph.tile([128, 512], bf16, tag="ysb")
                        nc.vector.tensor_copy(out=y_sb[:], in_=y_ps[:])
                        nc.sync.dma_start(
                            out=ypart[mi * 128:(mi + 1) * 128,
                                      nh * 512:(nh + 1) * 512],
                            in_=y_sb[:])

                # -------------- phase D: pair reduce + output --------------
                nc.gpsimd.collective_compute(
                    "ReduceScatter", mybir.AluOpType.add,
                    replica_groups=PAIRS,
                    ins=[ypart[:].opt()], outs=[yhalf[:].opt()])
                nc.sync.dma_start(out=y[:], in_=yhalf[:])

    import concourse.mybir as mybir2
    _split_sync_waits(nc, mybir2, max_waits=1)
    return nc


def _make_in_maps(q, kv, Wq, Wkv, Wc, bias):
    import ml_dtypes
    bf = ml_dtypes.bfloat16

    # int16 fixed-point for the score path; scales fold into kT2 below
    sq = float(np.abs(q).max()) / QSCALE
    sw = float(np.abs(Wq).max()) / QSCALE
    qi = np.round(q * (1.0 / sq)).astype(np.int16)          # [B, L, D]
    wi = np.round(Wq * (1.0 / sw)).astype(np.int16)         # [D, D]

    # host-side f32 kv projection + l2 norm over seq:
    # k pre-scaled by sq*sw/sqrt(dh), v shipped bf16
    kvp = np.einsum("bld,kd->blk", kv, Wkv, optimize=True)  # [B, L, 128]
    n = np.sqrt((kvp * kvp).sum(axis=1, keepdims=True))
    kvn = kvp / np.maximum(n, 1e-12)
    kvnT = kvn.transpose(0, 2, 1)                           # [B, 128, L]
    kT2 = (0.125 * sq * sw) * np.ascontiguousarray(kvnT[:, :DH])
    vT = np.ascontiguousarray(kvnT[:, DH:]).astype(bf)      # [B, 64, L]

    g = 1.0 / (1.0 + np.exp(-bias.astype(np.float64)))
    grow = np.zeros(1024, np.float32)
    grow[128:256:2] = g
    grow[129:256:2] = 1.0 - g

    blob32 = np.zeros((8, B32_R, 1024), np.float32)
    blob16 = np.zeros((8, B16_R, 1024), bf)
    for bi in range(B):
        c0, c1 = 2 * bi, 2 * bi + 1
        # f32 pair content: kT2 0:64 | gates 64 | pad 65
        blob32[c0, 0:P32_SH] = kT2[bi][0:P32_SH]
        blob32[c1, 0:64 - P32_SH] = kT2[bi][P32_SH:64]
        blob32[c1, P32_G - P32_SH] = grow
        # bf16 pair content: vT 0:64 | q int16 bits 64:1088
        qb = qi[bi].view(bf)                                # [L, D] bits
        blob16[c0, 0:64] = vT[bi]
        blob16[c0, 64:P16_SH] = qb[0:P16_SH - 64]
        blob16[c1, 0:P16_R - P16_SH] = qb[P16_SH - 64:L]

    wib = np.ascontiguousarray(wi.T).view(bf)               # [D, D] bits
    for hg in range(2):
        sl = slice(hg * CPG, (hg + 1) * CPG)
        e16 = np.empty((E16_R, 1024), bf)
        e16[E16_WC:E16_WC + 512] = np.ascontiguousarray(
            Wc[:, sl].T).astype(bf)
        e16[E16_WQ:E16_WQ + 512] = np.ascontiguousarray(
            wib[:, sl]).reshape(512, 1024)
        for j in range(4):
            c = 2 * j + hg
            blob16[c, P16_SH:B16_R] = e16[j * E16_SH:(j + 1) * E16_SH]

    return [{"blob32": blob32[c], "blob16": blob16[c]} for c in range(8)]


def kernel(q, kv, Wq, Wkv, Wc, bias):
    from concourse.bass_utils import run_bass_kernel_spmd

    if "nc" not in _CACHE:
        _CACHE["nc"] = _build_nc()
    nc = _CACHE["nc"]

    in_maps = _make_in_maps(q, kv, Wq, Wkv, Wc, bias)
    res = run_bass_kernel_spmd(nc, in_maps, list(range(8)))
    out = np.empty((B, L, D), np.float32)
    for bi in range(B):
        out[bi, 0:512] = res.results[2 * bi]["y"].astype(np.float32)
        out[bi, 512:1024] = res.results[2 * bi + 1]["y"].astype(np.float32)
    return out


# revision 8
# speedup vs baseline: 3.7671x; 1.0538x over previous
"""KNN attention kernel for 8 Trainium2 NeuronCores.

Sharding: (batch, head-group) data parallel — core c handles batch c//2 and
heads (c%2)*8..(c%2)*8+8.  To minimize axon-tunnel traffic (the wall-clock
bottleneck), each core uploads a unique 1/8 shard of the inputs; on-device
AllGathers redistribute: pair gathers [[0,1],[2,3],..] for per-batch data and
an even/odd gather [[0,2,4,6],[1,3,5,7]] for per-head-group weight halves.

Precision scheme: the retrieval argmax is exquisitely sensitive to score
noise (bf16 inputs cause ~350 index flips → 4e-2 rel err; int16 fixed-point
causes only 4).  q and Wq ship as int16 fixed-point (converted to f32 on
device; the quantization scales fold into the host-shipped keys
kT2 = 0.125*sq*sw*l2norm(k).T, since argmax is scale-invariant and the
softmax scores see the product fold — no runtime scales needed on device).
The value path ships as a host-projected bf16 vT = l2norm(kv @ Wkv_v.T).T
(1 GFLOP on host, saves shipping kv/Wkv), Wc and the output y are bf16.
Partial final projections are pair-ReduceScattered on device so each core
outputs a disjoint bf16 [512, 1024] slice of y.
"""

import sys

sys.path.insert(0, "/opt/trn_rl_repo")

import numpy as np

B, L, D, DH, H = 4, 1024, 1024, 64, 16
HPG = 8          # heads per core
CPG = HPG * DH   # channels per core (512)

# single bf16-typed blob.  pair content rows: [vT 0:64 | kT2 f32-bits
# 64:192 | gates f32-bits 192 | q int16 bits 193:1217 | pad 1217]; eo
# content rows: [wcT half 0:512 | wqT-half int16 bits flat 512:1024]
P16_V, P16_KT, P16_G, P16_Q = 0, 64, 192, 193
P16_R, P16_SH = 1218, 609
E16_WC, E16_WQ = 0, 512
E16_R, E16_SH = 1024, 256
B16_R = P16_SH + E16_SH  # 865

QSCALE = 32766.0

_CACHE = {}


def _split_sync_waits(nc, mybir, max_waits=1):
    """This container's walrus rejects >1 sync wait per instruction; spill
    extras onto same-engine NOPs placed immediately before."""
    for fn in nc.m.functions:
        for bb in fn.blocks:
            old = list(bb.instructions)
            new_insts = []
            changed = False
            for inst in old:
                si = inst.sync_info
                if si is not None and len(si.on_wait) > max_waits:
                    waits = list(si.on_wait)
                    extra, keep = waits[:-max_waits], waits[-max_waits:]
                    k = 0
                    while extra:
                        chunk, extra = extra[:max_waits], extra[max_waits:]
                        nop = mybir.InstNoOp(
                            name=f"{inst.name}_ws{k}", ins=[], outs=[])
                        nop.engine = inst.engine
                        nop.sync_info = mybir.SyncInfo(
                            on_wait=chunk, on_update=[])
                        nc.register_instruction(nop)
                        new_insts.append(nop)
                        k += 1
                    inst.sync_info = mybir.SyncInfo(
                        on_wait=keep, on_update=list(si.on_update))
                    changed = True
                new_insts.append(inst)
            if changed:
                bb.instructions = new_insts


def _build_nc():
    import concourse.bass as bass
    import concourse.mybir as mybir
    import concourse.tile as tile
    from concourse.masks import make_identity

    f32 = mybir.dt.float32
    bf16 = mybir.dt.bfloat16
    i16 = mybir.dt.int16
    u32 = mybir.dt.uint32
    Exp = mybir.ActivationFunctionType.Exp
    mul_op = mybir.AluOpType.mult

    nc = bass.Bass("TRN2", target_bir_lowering=False, debug=False,
                   num_devices=8)

    blob16 = nc.dram_tensor("blob16", [B16_R, 1024], bf16,
                            kind="ExternalInput")
    y = nc.dram_tensor("y", [512, D], bf16, kind="ExternalOutput")

    k_nat_d = nc.dram_tensor("k_nat_d", [L, DH], bf16)
    v_ret_d = nc.dram_tensor("v_ret_d", [L, DH], bf16)

    PAIRS = [[0, 1], [2, 3], [4, 5], [6, 7]]
    EODD = [[0, 2, 4, 6], [1, 3, 5, 7]]

    with tile.TileContext(nc) as tc:
        with (
            tc.tile_pool(name="dram", bufs=1, space="DRAM") as dp,
            tc.tile_pool(name="persist", bufs=1) as pw,
            tc.tile_pool(name="psbig", bufs=2, space="PSUM") as ps_big,
            tc.tile_pool(name="psav", bufs=2, space="PSUM") as ps_av,
            tc.tile_pool(name="pssm", bufs=2, space="PSUM") as ps_sm,
        ):
            # ---------------- phase 0: gather shards ----------------
            p16_in = dp.tile([P16_SH, 1024], bf16)
            p16_full = dp.tile([P16_R, 1024], bf16)
            e16_in = dp.tile([E16_SH, 1024], bf16)
            e16_full = dp.tile([E16_R, 1024], bf16)
            ypart = dp.tile([L, D], bf16)
            yhalf = dp.tile([512, D], bf16)

            nc.sync.dma_start(out=p16_in[:], in_=blob16[0:P16_SH, :])
            nc.sync.dma_start(out=e16_in[:], in_=blob16[P16_SH:B16_R, :])
            nc.gpsimd.collective_compute(
                "AllGather", mybir.AluOpType.bypass, replica_groups=PAIRS,
                ins=[p16_in[:].opt()], outs=[p16_full[:].opt()])
            nc.gpsimd.collective_compute(
                "AllGather", mybir.AluOpType.bypass, replica_groups=EODD,
                ins=[e16_in[:].opt()], outs=[e16_full[:].opt()])

            ident_bf = pw.tile([128, 128], bf16)
            make_identity(nc, ident_bf[:])
            ones_sb = pw.tile([128, 64], f32)
            nc.vector.memset(ones_sb[:], 1.0)

            # gates [128, 2] f32 <- one interleaved f32 row
            gates_sb = pw.tile([128, 2], f32)
            nc.sync.dma_start(
                out=gates_sb[:],
                in_=p16_full[P16_G:P16_G + 1, 0:512].bitcast(f32))

            # wc_sb[:, cc, :] = wcT rows (hg half, device-invariant layout)
            wc_sb = pw.tile([128, 4, D], bf16)
            for cc in range(4):
                nc.sync.dma_start(
                    out=wc_sb[:, cc, :],
                    in_=e16_full[E16_WC + cc * 128:E16_WC + (cc + 1) * 128, :])

            qpT_f = pw.tile([128, 4, L], f32)     # [c=512, i] c=cc*128+p
            qpT_b = pw.tile([128, 4, L], bf16)
            kT2_f = pw.tile([128, L], f32)        # rows 0:64 kT, 64:128 dup
            kT2_b = pw.tile([128, L], bf16)
            vloc_T = pw.tile([128, L], bf16)      # rows 64:128 used
            vret_T = pw.tile([128, L], bf16)      # rows 64:128 used
            vloc_nat = pw.tile([128, 8, DH + 1], bf16)
            attnT = pw.tile([128, 4, L], bf16)    # [c=512, i]

            # kT2: shipped f32, pre-scaled by sq*sw/sqrt(dh); dup row halves
            nc.sync.dma_start(
                out=kT2_f[0:64, :],
                in_=p16_full[P16_KT:P16_KT + 128, :].bitcast(f32))
            nc.sync.dma_start(out=kT2_f[64:128, :], in_=kT2_f[0:64, :])
            nc.vector.tensor_copy(out=kT2_b[:], in_=kT2_f[:])

            # vT: shipped bf16, load to partitions 64:128; fold gates
            vT_sb = pw.tile([128, L], bf16)
            nc.sync.dma_start(
                out=vT_sb[64:128, :], in_=p16_full[P16_V:P16_V + 64, :])
            nc.vector.tensor_scalar_mul(
                vloc_T[64:128, :], vT_sb[64:128, :], gates_sb[64:128, 1:2])
            nc.vector.tensor_scalar_mul(
                vret_T[64:128, :], vT_sb[64:128, :], gates_sb[64:128, 0:1])

            # ---------------- phase A: unpack + projections ----------------
            with tc.tile_pool(name="load", bufs=1) as pl, \
                 tc.tile_pool(name="worka", bufs=2) as wa:
                # q int16 bits: transpose-load then convert to f32
                qT_bits = pl.tile([128, 8, L], bf16)
                qT_sb = pl.tile([128, 8, L], f32)
                wq_bits = pl.tile([128, 8, CPG], bf16)
                wq_sb = pl.tile([128, 8, CPG], f32)
                for kc in range(8):
                    nc.sync.dma_start(
                        out=qT_bits[:, kc, :],
                        in_=p16_full[P16_Q:P16_Q + L, kc * 128:(kc + 1) * 128],
                        transpose=True)
                    # wqT half int16 stored flat: d-chunk kc = 64 blob rows
                    nc.sync.dma_start(
                        out=wq_bits[:, kc, :],
                        in_=e16_full[E16_WQ + kc * 64:E16_WQ + (kc + 1) * 64, :])
                nc.vector.tensor_copy(
                    out=qT_sb[:], in_=qT_bits[:].bitcast(i16))
                nc.vector.tensor_copy(
                    out=wq_sb[:], in_=wq_bits[:].bitcast(i16))

                # natural-layout copies: vloc (sbuf, +ones col), vret/k (dram)
                nc.vector.memset(vloc_nat[:, :, DH:DH + 1], 1.0)
                for jc in range(8):
                    tp = ps_sm.tile([128, 128], bf16, tag="sm")
                    nc.tensor.transpose(
                        out=tp[:, 0:64],
                        in_=vloc_T[64:128, jc * 128:(jc + 1) * 128],
                        identity=ident_bf[64:128, 64:128])
                    nc.vector.tensor_copy(
                        out=vloc_nat[:, jc, 0:DH], in_=tp[:, 0:64])

                    tp2 = ps_sm.tile([128, 128], bf16, tag="sm")
                    nc.tensor.transpose(
                        out=tp2[:, 0:64],
                        in_=vret_T[64:128, jc * 128:(jc + 1) * 128],
                        identity=ident_bf[64:128, 64:128])
                    vr = wa.tile([128, DH], bf16, tag="vr")
                    nc.vector.tensor_copy(out=vr[:], in_=tp2[:, 0:64])
                    nc.sync.dma_start(
                        out=v_ret_d[jc * 128:(jc + 1) * 128, :], in_=vr[:])

                    tp3 = ps_sm.tile([128, 128], bf16, tag="sm")
                    nc.tensor.transpose(
                        out=tp3[:, 0:64],
                        in_=kT2_b[0:64, jc * 128:(jc + 1) * 128],
                        identity=ident_bf[0:64, 0:64])
                    kn = wa.tile([128, DH], bf16, tag="kn")
                    nc.vector.tensor_copy(out=kn[:], in_=tp3[:, 0:64])
                    nc.sync.dma_start(
                        out=k_nat_d[jc * 128:(jc + 1) * 128, :], in_=kn[:])

                # q projection qpT[c, i] (values scaled by 1/(sq*sw))
                for cc in range(4):
                    for ic in range(2):
                        ps = ps_av.tile([128, 512], f32, tag="av")
                        for kc in range(8):
                            nc.tensor.matmul(
                                ps[:],
                                lhsT=wq_sb[:, kc, cc * 128:(cc + 1) * 128],
                                rhs=qT_sb[:, kc, ic * 512:(ic + 1) * 512],
                                start=(kc == 0), stop=(kc == 7))
                        sl = slice(ic * 512, (ic + 1) * 512)
                        nc.vector.tensor_copy(out=qpT_f[:, cc, sl], in_=ps[:])
                        nc.scalar.copy(out=qpT_b[:, cc, sl], in_=ps[:])

            # ---------------- phase B: per-head attention ----------------
            with tc.tile_pool(name="head", bufs=2) as ph:
                for h in range(HPG):
                    pb = (h % 2) * 64
                    cc = h // 2
                    qh_f = qpT_f[pb:pb + 64, cc, :]     # [64, L] f32 view
                    qh_b = qpT_b[pb:pb + 64, cc, :]     # [64, L] bf16 view

                    # --- scores S[i, j] (fp32) + argmax ---
                    idx8 = ph.tile([128, 8, 8], u32, tag="idx")
                    for qi in range(8):
                        s_ps = ps_big.tile([128, 1024], f32, tag="sbig")
                        for jh in range(2):
                            nc.tensor.matmul(
                                s_ps[:, jh * 512:(jh + 1) * 512],
                                lhsT=qh_f[:, qi * 128:(qi + 1) * 128],
                                rhs=kT2_f[pb:pb + 64, jh * 512:(jh + 1) * 512],
                                start=True, stop=True)
                        ssb = ph.tile([128, 1024], f32, tag="ssb")
                        nc.vector.tensor_copy(out=ssb[:], in_=s_ps[:])
                        m8 = ph.tile([128, 8], f32, tag="m8")
                        nc.vector.max(out=m8[:], in_=ssb[:])
                        nc.vector.max_index(
                            out=idx8[:, qi, :], in_max=m8[:], in_values=ssb[:])

                    # --- local: E = exp(S^T) ---
                    E1 = ph.tile([128, 8, 1024], bf16, tag="E1")
                    for jc in range(8):
                        st_ps = ps_big.tile([128, 1024], f32, tag="sbig")
                        for ih in range(2):
                            nc.tensor.matmul(
                                st_ps[:, ih * 512:(ih + 1) * 512],
                                lhsT=kT2_b[pb:pb + 64, jc * 128:(jc + 1) * 128],
                                rhs=qh_b[:, ih * 512:(ih + 1) * 512],
                                start=True, stop=True)
                        nc.scalar.activation(
                            out=E1[:, jc, :], in_=st_ps[:], func=Exp)

                    # --- gather retrieved k/v rows; build rkT (dup halves) ---
                    rkT = ph.tile([128, 1024], bf16, tag="rkT")
                    rv_nat = ph.tile([128, 8, DH + 1], bf16, tag="rvn")
                    nc.vector.memset(rv_nat[:, :, DH:DH + 1], 1.0)
                    for qi in range(8):
                        rk = ph.tile([128, DH], bf16, tag="rk")
                        nc.gpsimd.indirect_dma_start(
                            out=rk[:], out_offset=None,
                            in_=k_nat_d[:],
                            in_offset=bass.IndirectOffsetOnAxis(
                                ap=idx8[:, qi, 0:1], axis=0))
                        nc.gpsimd.indirect_dma_start(
                            out=rv_nat[:, qi, 0:DH], out_offset=None,
                            in_=v_ret_d[:],
                            in_offset=bass.IndirectOffsetOnAxis(
                                ap=idx8[:, qi, 0:1], axis=0))
                        tp = ps_sm.tile([128, 128], bf16, tag="sm")
                        nc.tensor.transpose(
                            out=tp[0:64, :], in_=rk[:],
                            identity=ident_bf[:, :])
                        nc.vector.tensor_copy(
                            out=rkT[0:64, qi * 128:(qi + 1) * 128],
                            in_=tp[0:64, :])
                    nc.sync.dma_start(
                        out=rkT[64:128, :], in_=rkT[0:64, :])

                    # --- retrieval: E2 = exp(S2^T) ---
                    E2 = ph.tile([128, 8, 1024], bf16, tag="E2")
                    for jc in range(8):
                        st_ps = ps_big.tile([128, 1024], f32, tag="sbig")
                        for ih in range(2):
                            nc.tensor.matmul(
                                st_ps[:, ih * 512:(ih + 1) * 512],
                                lhsT=rkT[pb:pb + 64, jc * 128:(jc + 1) * 128],
                                rhs=qh_b[:, ih * 512:(ih + 1) * 512],
                                start=True, stop=True)
                        nc.scalar.activation(
                            out=E2[:, jc, :], in_=st_ps[:], func=Exp)

                    # --- weighted sums + normalize + combine ---
                    attn_h = ph.tile([64, 1024], bf16, tag="ath")
                    for ic in range(2):
                        isl = slice(ic * 512, (ic + 1) * 512)
                        avL = ps_av.tile([65, 512], f32, tag="av")
                        avR = ps_av.tile([65, 512], f32, tag="av")
                        for jc in range(8):
                            nc.tensor.matmul(
                                avL[:], lhsT=vloc_nat[:, jc, :],
                                rhs=E1[:, jc, isl],
                                start=(jc == 0), stop=(jc == 7))
                        for jc in range(8):
                            nc.tensor.matmul(
                                avR[:], lhsT=rv_nat[:, jc, :],
                                rhs=E2[:, jc, isl],
                                start=(jc == 0), stop=(jc == 7))
                        rL = ph.tile([65, 512], f32, tag="rL")
                        rR = ph.tile([65, 512], f32, tag="rR")
                        nc.vector.reciprocal(out=rL[64:65, :], in_=avL[64:65, :])
                        nc.vector.reciprocal(out=rR[64:65, :], in_=avR[64:65, :])
                        bcL = ps_sm.tile([64, 512], f32, tag="sm")
                        bcR = ps_sm.tile([64, 512], f32, tag="sm")
                        nc.tensor.matmul(
                            bcL[:], lhsT=ones_sb[64:65, :], rhs=rL[64:65, :],
                            start=True, stop=True)
                        nc.tensor.matmul(
                            bcR[:], lhsT=ones_sb[64:65, :], rhs=rR[64:65, :],
                            start=True, stop=True)
                        bcLs = ph.tile([64, 512], f32, tag="bcLs")
                        bcRs = ph.tile([64, 512], f32, tag="bcRs")
                        nc.vector.tensor_copy(out=bcLs[:], in_=bcL[:])
                        nc.vector.tensor_copy(out=bcRs[:], in_=bcR[:])
                        bLs = ph.tile([64, 512], f32, tag="bLs")
                        bRs = ph.tile([64, 512], f32, tag="bRs")
                        nc.vector.tensor_tensor(
                            out=bLs[:], in0=avL[0:64, :], in1=bcLs[:], op=mul_op)
                        nc.vector.tensor_tensor(
                            out=bRs[:], in0=avR[0:64, :], in1=bcRs[:], op=mul_op)
                        nc.vector.tensor_add(
                            out=attn_h[:, isl], in0=bLs[:], in1=bRs[:])
                    nc.sync.dma_start(
                        out=attnT[pb:pb + 64, cc, :], in_=attn_h[:])

                # ---------------- phase C: output projection ----------------
                for mi in range(8):
                    for nh in range(2):
                        y_ps = ps_av.tile([128, 512], f32, tag="av")
                        for cc2 in range(4):
                            nc.tensor.matmul(
                                y_ps[:],
                                lhsT=attnT[:, cc2, mi * 128:(mi + 1) * 128],
                                rhs=wc_sb[:, cc2, nh * 512:(nh + 1) * 512],
                                start=(cc2 == 0), stop=(cc2 == 3))
                        y_sb = ph.tile([128, 512], bf16, tag="ysb")
                        nc.vector.tensor_copy(out=y_sb[:], in_=y_ps[:])
                        nc.sync.dma_start(
                            out=ypart[mi * 128:(mi + 1) * 128,
                                      nh * 512:(nh + 1) * 512],
                            in_=y_sb[:])

                # -------------- phase D: pair reduce + output --------------
                nc.gpsimd.collective_compute(
                    "ReduceScatter", mybir.AluOpType.add,
                    replica_groups=PAIRS,
                    ins=[ypart[:].opt()], outs=[yhalf[:].opt()])
                nc.sync.dma_start(out=y[:], in_=yhalf[:])

    import concourse.mybir as mybir2
    _split_sync_waits(nc, mybir2, max_waits=1)
    return nc


def _make_in_maps(q, kv, Wq, Wkv, Wc, bias):
    import ml_dtypes
    bf = ml_dtypes.bfloat16

    # int16 fixed-point for the score path; scales fold into kT2 below
    sq = float(np.abs(q).max()) / QSCALE
    sw = float(np.abs(Wq).max()) / QSCALE
    qi = np.round(q * (1.0 / sq)).astype(np.int16)          # [B, L, D]
    wi = np.round(Wq * (1.0 / sw)).astype(np.int16)         # [D, D]

    # host-side f32 kv projection + l2 norm over seq:
    # k pre-scaled by sq*sw/sqrt(dh), v shipped bf16
    kvp = np.einsum("bld,kd->blk", kv, Wkv, optimize=True)  # [B, L, 128]
    n = np.sqrt((kvp * kvp).sum(axis=1, keepdims=True))
    kvn = kvp / np.maximum(n, 1e-12)
    kvnT = kvn.transpose(0, 2, 1)                           # [B, 128, L]
    kT2 = (0.125 * sq * sw) * np.ascontiguousarray(kvnT[:, :DH])
    vT = np.ascontiguousarray(kvnT[:, DH:]).astype(bf)      # [B, 64, L]

    g = 1.0 / (1.0 + np.exp(-bias.astype(np.float64)))
    grow = np.zeros(1024, np.float32)
    grow[128:256:2] = g
    grow[129:256:2] = 1.0 - g

    growb = grow.view(bf)                                   # [2048] bits
    blob16 = np.zeros((8, B16_R, 1024), bf)
    for bi in range(B):
        c0, c1 = 2 * bi, 2 * bi + 1
        # pair content: vT | kT2 bits | gates bits | q int16 bits
        qb = qi[bi].view(bf)                                # [L, D] bits
        blob16[c0, 0:64] = vT[bi]
        blob16[c0, P16_KT:P16_KT + 128] = kT2[bi].view(bf).reshape(128, 1024)
        blob16[c0, P16_G, 0:512] = growb[0:512]
        blob16[c0, P16_Q:P16_SH] = qb[0:P16_SH - P16_Q]
        blob16[c1, 0:1024 - (P16_SH - P16_Q)] = qb[P16_SH - P16_Q:L]

    wib = np.ascontiguousarray(wi.T).view(bf)               # [D, D] bits
    for hg in range(2):
        sl = slice(hg * CPG, (hg + 1) * CPG)
        e16 = np.empty((E16_R, 1024), bf)
        e16[E16_WC:E16_WC + 512] = np.ascontiguousarray(
            Wc[:, sl].T).astype(bf)
        e16[E16_WQ:E16_WQ + 512] = np.ascontiguousarray(
            wib[:, sl]).reshape(512, 1024)
        for j in range(4):
            c = 2 * j + hg
            blob16[c, P16_SH:B16_R] = e16[j * E16_SH:(j + 1) * E16_SH]

    return [{"blob16": blob16[c]} for c in range(8)]


def kernel(q, kv, Wq, Wkv, Wc, bias):
    from concourse.bass_utils import run_bass_kernel_spmd

    if "nc" not in _CACHE:
        _CACHE["nc"] = _build_nc()
    nc = _CACHE["nc"]

    in_maps = _make_in_maps(q, kv, Wq, Wkv, Wc, bias)
    res = run_bass_kernel_spmd(nc, in_maps, list(range(8)))
    out = np.empty((B, L, D), np.float32)
    for bi in range(B):
        out[bi, 0:512] = res.results[2 * bi]["y"].astype(np.float32)
        out[bi, 512:1024] = res.results[2 * bi + 1]["y"].astype(np.float32)
    return out
